# revision 8
# baseline (speedup 1.0000x reference)
"""Trainium2 Bass kernel for nn_AttnResBlock (B=16, C=512, A=64, L=1024).

Data-parallel over batch: 8 cores x 2 batches each. Weights replicated.
BatchNorm (training mode, stats over (B, L)) needs global batch stats ->
two tiny [128, 8] f32 AllReduces (local mean / E[x^2] per channel).

All matmuls run as float32r (TF32-like, 1 cycle/row vs 4 for fp32).
Layouts chosen so no on-chip transposes are needed:
  - x      [b, c, l]  (c on partitions)  : kq-matmul rhs, residual
  - xT     [b, l, c]  (host-transposed)  : attn-output lhsT
  - keys2/queries2 [a2, l] with batch 0 in partitions 0:64, batch 1 in
    64:128 -> scores for both batches via PE row/col tiling.
"""
import numpy as np

P = 128
B, C, A, L = 16, 512, 64, 1024
NCORES = 8
BL = B // NCORES          # local batches per core
CT = C // P               # 4 channel tiles
LT = L // P               # 8 length tiles
MC = L // 512             # 2 moving chunks
EPS = 1e-5
SM_SCALE = 2.0 / L        # softmax scale: scores/(L/2)

_CACHE = {}


def _build():
    import concourse.bass as bass
    import concourse.mybir as mybir
    from concourse import bacc
    from concourse.tile import TileContext

    f32 = mybir.dt.float32
    f32r = mybir.dt.float32r
    AF = mybir.ActivationFunctionType
    ALU = mybir.AluOpType

    nc = bacc.Bacc(num_devices=NCORES)

    x_ext = nc.declare_dram_parameter("x", [BL, C, L], f32r, isOutput=False)
    xT_ext = nc.declare_dram_parameter("xT", [BL, L, C], f32, isOutput=False)
    wk_ext = nc.declare_dram_parameter("wk", [C, A], f32r, isOutput=False)
    wq_ext = nc.declare_dram_parameter("wq", [C, A], f32r, isOutput=False)
    bk2_ext = nc.declare_dram_parameter("bk2", [P, 1], f32, isOutput=False)
    bq2_ext = nc.declare_dram_parameter("bq2", [P, 1], f32, isOutput=False)
    wp_ext = nc.declare_dram_parameter("wp", [C, C], f32r, isOutput=False)
    bp_ext = nc.declare_dram_parameter("bp", [C, 1], f32, isOutput=False)
    w1_ext = nc.declare_dram_parameter("w1", [3, C, C], f32r, isOutput=False)
    b1_ext = nc.declare_dram_parameter("b1", [C, 1], f32, isOutput=False)
    w2_ext = nc.declare_dram_parameter("w2", [3, C, C], f32r, isOutput=False)
    b2_ext = nc.declare_dram_parameter("b2", [C, 1], f32, isOutput=False)
    g1_ext = nc.declare_dram_parameter("g1", [C, 1], f32, isOutput=False)
    be1_ext = nc.declare_dram_parameter("be1", [C, 1], f32, isOutput=False)
    g2_ext = nc.declare_dram_parameter("g2", [C, 1], f32, isOutput=False)
    be2_ext = nc.declare_dram_parameter("be2", [C, 1], f32, isOutput=False)
    out_ext = nc.declare_dram_parameter("out", [BL, C, L], f32, isOutput=True)

    cc1_in = nc.dram_tensor("cc1_in", [P, 2 * CT], f32)
    cc1_out = nc.dram_tensor("cc1_out", [P, 2 * CT], f32, addr_space="Shared")
    cc2_in = nc.dram_tensor("cc2_in", [P, 2 * CT], f32)
    cc2_out = nc.dram_tensor("cc2_out", [P, 2 * CT], f32, addr_space="Shared")

    rg = [list(range(NCORES))]

    with TileContext(nc) as tc:
        with tc.tile_pool(name="persist", bufs=1) as pers, \
             tc.tile_pool(name="small", bufs=1) as small, \
             tc.tile_pool(name="ostage", bufs=4) as ostage, \
             tc.tile_pool(name="psum", bufs=8, space="PSUM") as psum:

            x2_sb = pers.tile([P, BL, CT, L], f32)

            # per-channel parameter vectors -> [P, CT] layout
            def load_vec(ext, tag):
                t = small.tile([P, CT], f32, tag=tag)
                for ct in range(CT):
                    nc.gpsimd.dma_start(out=t[:, ct : ct + 1],
                                        in_=ext[ct * P:(ct + 1) * P, 0:1])
                return t

            bp_sb = load_vec(bp_ext, "bp")
            b1_sb = load_vec(b1_ext, "b1")
            b2_sb = load_vec(b2_ext, "b2")
            g1_sb = load_vec(g1_ext, "g1")
            be1_sb = load_vec(be1_ext, "be1")
            g2_sb = load_vec(g2_ext, "g2")
            be2_sb = load_vec(be2_ext, "be2")
            bk2_sb = small.tile([P, 1], f32, tag="bk2")
            nc.gpsimd.dma_start(out=bk2_sb[:], in_=bk2_ext[:])
            bq2_sb = small.tile([P, 1], f32, tag="bq2")
            nc.gpsimd.dma_start(out=bq2_sb[:], in_=bq2_ext[:])

            ccin1_sb = small.tile([P, 2 * CT], f32, tag="ccin1")
            ccout1_sb = small.tile([P, 2 * CT], f32, tag="ccout1")
            ccin2_sb = small.tile([P, 2 * CT], f32, tag="ccin2")
            ccout2_sb = small.tile([P, 2 * CT], f32, tag="ccout2")
            scale1 = small.tile([P, CT], f32, tag="scale1")
            bias1 = small.tile([P, CT], f32, tag="bias1")
            scale2 = small.tile([P, CT], f32, tag="scale2")
            bias2 = small.tile([P, CT], f32, tag="bias2")
            eps_sb = small.tile([P, 1], f32, tag="eps")
            nc.vector.memset(eps_sb[:], EPS)

            # ---------------- Phase A: attention ----------------
            with tc.tile_pool(name="phA", bufs=1) as pa, \
                 tc.tile_pool(name="phAb", bufs=1) as pab:
                x_sb = pa.tile([P, BL, CT, L], f32r)
                for b in range(BL):
                    for ct in range(CT):
                        nc.sync.dma_start(out=x_sb[:, b, ct, :],
                                          in_=x_ext[b, ct * P:(ct + 1) * P, :])
                wk_sb = pa.tile([P, CT, A], f32r)
                wq_sb = pa.tile([P, CT, A], f32r)
                wp_sb = pa.tile([P, CT, C], f32r)
                for ct in range(CT):
                    nc.sync.dma_start(out=wk_sb[:, ct, :], in_=wk_ext[ct * P:(ct + 1) * P, :])
                    nc.sync.dma_start(out=wq_sb[:, ct, :], in_=wq_ext[ct * P:(ct + 1) * P, :])
                    nc.sync.dma_start(out=wp_sb[:, ct, :], in_=wp_ext[ct * P:(ct + 1) * P, :])

                keys_sb = pa.tile([P, BL, L], f32r)     # partitions 0:64 used
                queries_sb = pa.tile([P, BL, L], f32r)
                for dst, w_sb, bias_sb in ((keys_sb, wk_sb, bk2_sb), (queries_sb, wq_sb, bq2_sb)):
                    for b in range(BL):
                        for mc in range(MC):
                            ms = slice(mc * 512, (mc + 1) * 512)
                            kps = psum.tile([P, 512], f32, tag="ps")
                            for ct in range(CT):
                                nc.tensor.matmul(
                                    out=kps[0:A, :],
                                    lhsT=w_sb[:, ct, :],
                                    rhs=x_sb[:, b, ct, ms],
                                    start=(ct == 0), stop=(ct == CT - 1))
                            nc.vector.tensor_scalar_add(out=dst[0:A, b, ms],
                                                        in0=kps[0:A, :],
                                                        scalar1=bias_sb[0:A])

                for b in range(BL):
                    xT_sb = pab.tile([P, LT, C], f32, tag="xT")
                    for lc in range(LT):
                        nc.sync.dma_start(out=xT_sb[:, lc, :],
                                          in_=xT_ext[b, lc * P:(lc + 1) * P, :])

                    e_sb = pab.tile([P, LT, L], f32r, tag="e")
                    rsp = pab.tile([P, LT, MC], f32, tag="rsp")
                    for lc in range(LT):
                        for mc in range(MC):
                            sps = psum.tile([P, 512], f32, tag="ps")
                            nc.tensor.matmul(
                                out=sps[:],
                                lhsT=keys_sb[0:A, b, lc * P:(lc + 1) * P],
                                rhs=queries_sb[0:A, b, mc * 512:(mc + 1) * 512],
                                start=True, stop=True)
                            nc.scalar.activation(
                                out=e_sb[:, lc, mc * 512:(mc + 1) * 512],
                                in_=sps[:], func=AF.Exp, scale=SM_SCALE,
                                accum_out=rsp[:, lc, mc:mc + 1])
                    rs = pab.tile([P, LT], f32, tag="rs")
                    nc.vector.tensor_add(out=rs[:], in0=rsp[:, :, 0], in1=rsp[:, :, 1])
                    rcp = pab.tile([P, LT], f32, tag="rcp")
                    nc.vector.reciprocal(out=rcp[:], in_=rs[:])

                    # xTs[l, c] = xT[l, c] / rowsum[l]  (softmax denom folded in)
                    xTs = pab.tile([P, LT, C], f32r, tag="xTs")
                    for lc in range(LT):
                        nc.vector.tensor_scalar_mul(out=xTs[:, lc, :],
                                                    in0=xT_sb[:, lc, :],
                                                    scalar1=rcp[:, lc:lc + 1])

                    ao_sb = pab.tile([P, CT, L], f32r, tag="ao")
                    for cc in range(CT):
                        for mc in range(MC):
                            aps = psum.tile([P, 512], f32, tag="ps")
                            for lc in range(LT):
                                nc.tensor.matmul(
                                    out=aps[:],
                                    lhsT=xTs[:, lc, cc * P:(cc + 1) * P],
                                    rhs=e_sb[:, lc, mc * 512:(mc + 1) * 512],
                                    start=(lc == 0), stop=(lc == LT - 1))
                            nc.scalar.activation(out=ao_sb[:, cc, mc * 512:(mc + 1) * 512],
                                                 in_=aps[:], func=AF.Copy)

                    for oc in range(CT):
                        for mc in range(MC):
                            ms = slice(mc * 512, (mc + 1) * 512)
                            pps = psum.tile([P, 512], f32, tag="ps")
                            for ct in range(CT):
                                nc.tensor.matmul(
                                    out=pps[:],
                                    lhsT=wp_sb[:, ct, oc * P:(oc + 1) * P],
                                    rhs=ao_sb[:, ct, ms],
                                    start=(ct == 0), stop=(ct == CT - 1))
                            # x2 = proj + bp + x
                            nc.vector.scalar_tensor_tensor(
                                out=x2_sb[:, b, oc, ms], in0=pps[:],
                                scalar=bp_sb[:, oc:oc + 1],
                                in1=x_sb[:, b, oc, ms].bitcast(f32),
                                op0=ALU.add, op1=ALU.add)

            # ---------------- BN1 stats + AllReduce ----------------
            def bn_partial_stats(src_sb, ccin_sb):
                for ct in range(CT):
                    st = small.tile([P, 2 * BL, 6], f32, tag="bnst")
                    i = 0
                    for b in range(BL):
                        for hc in range(MC):
                            nc.vector.bn_stats(out=st[:, i, :],
                                               in_=src_sb[:, b, ct, hc * 512:(hc + 1) * 512])
                            i += 1
                    mv = small.tile([P, 2], f32, tag="bnmv")
                    nc.vector.bn_aggr(out=mv[:], in_=st[:])
                    nc.vector.tensor_copy(out=ccin_sb[:, 2 * ct:2 * ct + 1], in_=mv[:, 0:1])
                    # E[x^2]_local = mean^2 + var
                    nc.vector.scalar_tensor_tensor(
                        out=ccin_sb[:, 2 * ct + 1:2 * ct + 2], in0=mv[:, 0:1],
                        scalar=mv[:, 0:1], in1=mv[:, 1:2],
                        op0=ALU.mult, op1=ALU.add)

            def bn_post(ccout_sb, g_sb, be_sb, scale_t, bias_t, tag):
                mg = small.tile([P, CT], f32, tag=tag + "mg")
                ex2 = small.tile([P, CT], f32, tag=tag + "ex2")
                nc.vector.tensor_scalar_mul(out=mg[:], in0=ccout_sb[:, 0::2],
                                            scalar1=1.0 / NCORES)
                nc.vector.tensor_scalar_mul(out=ex2[:], in0=ccout_sb[:, 1::2],
                                            scalar1=1.0 / NCORES)
                nvar = small.tile([P, CT], f32, tag=tag + "nv")
                # nvar = mean^2 - E[x^2] = -var
                nc.vector.tensor_tensor(out=nvar[:], in0=mg[:], in1=mg[:], op=ALU.mult)
                nc.vector.tensor_tensor(out=nvar[:], in0=nvar[:], in1=ex2[:], op=ALU.subtract)
                sd = small.tile([P, CT], f32, tag=tag + "sd")
                nc.scalar.activation(out=sd[:], in_=nvar[:], func=AF.Sqrt,
                                     scale=-1.0, bias=eps_sb[:])
                rstd = small.tile([P, CT], f32, tag=tag + "rstd")
                nc.vector.reciprocal(out=rstd[:], in_=sd[:])
                nc.vector.tensor_tensor(out=scale_t[:], in0=rstd[:], in1=g_sb[:], op=ALU.mult)
                tmp = small.tile([P, CT], f32, tag=tag + "tmp")
                nc.vector.tensor_tensor(out=tmp[:], in0=mg[:], in1=scale_t[:], op=ALU.mult)
                nc.vector.tensor_tensor(out=bias_t[:], in0=be_sb[:], in1=tmp[:], op=ALU.subtract)

            bn_partial_stats(x2_sb, ccin1_sb)

            with tc.tile_pool(name="phB", bufs=1) as pb:
                w1_sb = pb.tile([P, 3 * CT, C], f32r)
                for k in range(3):
                    for ct in range(CT):
                        nc.sync.dma_start(out=w1_sb[:, k * CT + ct, :],
                                          in_=w1_ext[k, ct * P:(ct + 1) * P, :])

                nc.gpsimd.dma_start(out=cc1_in[:], in_=ccin1_sb[:])
                nc.gpsimd.collective_compute(
                    "AllReduce", mybir.AluOpType.add, replica_groups=rg,
                    ins=[cc1_in[:].opt()], outs=[cc1_out[:].opt()])
                nc.gpsimd.dma_start(out=ccout1_sb[:], in_=cc1_out[:])
                bn_post(ccout1_sb, g1_sb, be1_sb, scale1, bias1, "p1")

                # h = relu(bn1(x2)), stored padded: h[:, b, ct, 1+l], zeros at 0 and L+1
                h_sb = pb.tile([P, BL, CT, L + 2], f32r, tag="hpad")
                # f32r memset is rejected by the ISA; write the pad zeros via ACT
                zsrc = eps_sb[:, 0:1].to_broadcast((P, BL, CT))
                nc.scalar.activation(out=h_sb[:, :, :, 0], in_=zsrc,
                                     func=AF.Copy, scale=0.0)
                nc.scalar.activation(out=h_sb[:, :, :, L + 1], in_=zsrc,
                                     func=AF.Copy, scale=0.0)
                for b in range(BL):
                    for ct in range(CT):
                        nc.scalar.activation(out=h_sb[:, b, ct, 1:L + 1],
                                             in_=x2_sb[:, b, ct, :], func=AF.Relu,
                                             scale=scale1[:, ct:ct + 1],
                                             bias=bias1[:, ct:ct + 1])

                w2_sb = pb.tile([P, 3 * CT, C], f32r)
                for k in range(3):
                    for ct in range(CT):
                        nc.sync.dma_start(out=w2_sb[:, k * CT + ct, :],
                                          in_=w2_ext[k, ct * P:(ct + 1) * P, :])

                # conv1: h2[o, l] = sum_{ct,k} w1[k][i, o].T @ h[i, l+k-1] + b1
                h2_sb = pb.tile([P, BL, CT, L], f32)
                for oc in range(CT):
                    cps = [psum.tile([P, 512], f32, tag="ps", name=f"cps{_j}") for _j in range(2 * BL)]
                    for ct in range(CT):
                        for k in range(3):
                            w_ap = w1_sb[:, k * CT + ct, oc * P:(oc + 1) * P]
                            for b in range(BL):
                                for hc in range(MC):
                                    nc.tensor.matmul(
                                        out=cps[2 * b + hc][:], lhsT=w_ap,
                                        rhs=h_sb[:, b, ct, hc * 512 + k:hc * 512 + k + 512],
                                        start=(ct == 0 and k == 0),
                                        stop=(ct == CT - 1 and k == 2))
                    for b in range(BL):
                        for hc in range(MC):
                            nc.vector.tensor_scalar_add(
                                out=h2_sb[:, b, oc, hc * 512:(hc + 1) * 512],
                                in0=cps[2 * b + hc][:], scalar1=b1_sb[:, oc:oc + 1])

                # BN2 stats + AllReduce
                bn_partial_stats(h2_sb, ccin2_sb)
                nc.gpsimd.dma_start(out=cc2_in[:], in_=ccin2_sb[:])
                nc.gpsimd.collective_compute(
                    "AllReduce", mybir.AluOpType.add, replica_groups=rg,
                    ins=[cc2_in[:].opt()], outs=[cc2_out[:].opt()])
                nc.gpsimd.dma_start(out=ccout2_sb[:], in_=cc2_out[:])
                bn_post(ccout2_sb, g2_sb, be2_sb, scale2, bias2, "p2")

                # h3 = relu(bn2(h2)) overwrites h_sb in place (pad zeros kept)
                for b in range(BL):
                    for ct in range(CT):
                        nc.scalar.activation(out=h_sb[:, b, ct, 1:L + 1],
                                             in_=h2_sb[:, b, ct, :], func=AF.Relu,
                                             scale=scale2[:, ct:ct + 1],
                                             bias=bias2[:, ct:ct + 1])

                # conv2 + b2 + residual(x2) -> out
                for oc in range(CT):
                    cps = [psum.tile([P, 512], f32, tag="ps", name=f"cps{_j}") for _j in range(2 * BL)]
                    for ct in range(CT):
                        for k in range(3):
                            w_ap = w2_sb[:, k * CT + ct, oc * P:(oc + 1) * P]
                            for b in range(BL):
                                for hc in range(MC):
                                    nc.tensor.matmul(
                                        out=cps[2 * b + hc][:], lhsT=w_ap,
                                        rhs=h_sb[:, b, ct, hc * 512 + k:hc * 512 + k + 512],
                                        start=(ct == 0 and k == 0),
                                        stop=(ct == CT - 1 and k == 2))
                    for b in range(BL):
                        for hc in range(MC):
                            og = ostage.tile([P, 512], f32, tag="og")
                            nc.vector.scalar_tensor_tensor(
                                out=og[:], in0=cps[2 * b + hc][:],
                                scalar=b2_sb[:, oc:oc + 1],
                                in1=x2_sb[:, b, oc, hc * 512:(hc + 1) * 512],
                                op0=ALU.add, op1=ALU.add)
                            nc.sync.dma_start(
                                out=out_ext[b, oc * P:(oc + 1) * P, hc * 512:(hc + 1) * 512],
                                in_=og[:])

    nc.compile()
    return nc


def _get_nc():
    if "nc" not in _CACHE:
        _CACHE["nc"] = _build()
    return _CACHE["nc"]


def _prep_in_maps(inputs):
    f = np.float32
    x = np.ascontiguousarray(inputs["x"], dtype=f)
    shared = {
        "wk": np.ascontiguousarray(inputs["Wk"].T, dtype=f),
        "wq": np.ascontiguousarray(inputs["Wq"].T, dtype=f),
        "bk2": np.concatenate([inputs["bk"], inputs["bk"]]).reshape(P, 1).astype(f),
        "bq2": np.concatenate([inputs["bq"], inputs["bq"]]).reshape(P, 1).astype(f),
        "wp": np.ascontiguousarray(inputs["Wp"].T, dtype=f),
        "bp": np.asarray(inputs["bp"], dtype=f).reshape(C, 1),
        "w1": np.ascontiguousarray(np.transpose(inputs["W1"], (2, 1, 0)), dtype=f),
        "b1": np.asarray(inputs["b1"], dtype=f).reshape(C, 1),
        "w2": np.ascontiguousarray(np.transpose(inputs["W2"], (2, 1, 0)), dtype=f),
        "b2": np.asarray(inputs["b2"], dtype=f).reshape(C, 1),
        "g1": np.asarray(inputs["g1"], dtype=f).reshape(C, 1),
        "be1": np.asarray(inputs["be1"], dtype=f).reshape(C, 1),
        "g2": np.asarray(inputs["g2"], dtype=f).reshape(C, 1),
        "be2": np.asarray(inputs["be2"], dtype=f).reshape(C, 1),
    }
    in_maps = []
    for i in range(NCORES):
        xl = np.ascontiguousarray(x[i * BL:(i + 1) * BL])
        xTl = np.ascontiguousarray(np.transpose(xl, (0, 2, 1)))
        m = {"x": xl, "xT": xTl}
        m.update(shared)
        in_maps.append(m)
    return in_maps


def kernel(**inputs) -> np.ndarray:
    from concourse import bass_utils
    nc = _get_nc()
    in_maps = _prep_in_maps(inputs)
    res = bass_utils.run_bass_kernel_spmd(nc, in_maps, list(range(NCORES)))
    return np.concatenate([r["out"] for r in res.results], axis=0)


# revision 11
# speedup vs baseline: 1.1243x; 1.1243x over previous
"""Trainium2 Bass kernel for nn_AttnResBlock (B=16, C=512, A=64, L=1024).

Data-parallel over batch: 8 cores x 2 batches each. Weights replicated.
BatchNorm (training mode, stats over (B, L)) needs global batch stats ->
two tiny [128, 8] f32 AllReduces (local mean / E[x^2] per channel).

All matmuls run as float32r (TF32-like, 1 cycle/row vs 4 for fp32).
Layouts chosen so no on-chip transposes are needed:
  - x      [b, c, l]  (c on partitions)  : kq-matmul rhs, residual
  - xT     [b, l, c]  (host-transposed)  : attn-output lhsT
  - keys2/queries2 [a2, l] with batch 0 in partitions 0:64, batch 1 in
    64:128 -> scores for both batches via PE row/col tiling.
"""
import numpy as np

P = 128
B, C, A, L = 16, 512, 64, 1024
NCORES = 8
BL = B // NCORES          # local batches per core
CT = C // P               # 4 channel tiles
LT = L // P               # 8 length tiles
MC = L // 512             # 2 moving chunks
EPS = 1e-5
SM_SCALE = 2.0 / L        # softmax scale: scores/(L/2)

_CACHE = {}


def _build():
    import concourse.bass as bass
    import concourse.mybir as mybir
    from concourse import bacc
    from concourse.tile import TileContext

    f32 = mybir.dt.float32
    f32r = mybir.dt.float32r
    AF = mybir.ActivationFunctionType
    ALU = mybir.AluOpType

    nc = bacc.Bacc(num_devices=NCORES)

    x_ext = nc.declare_dram_parameter("x", [BL, C, L], f32r, isOutput=False)
    xT_ext = nc.declare_dram_parameter("xT", [BL, L, C], f32, isOutput=False)
    wk_ext = nc.declare_dram_parameter("wk", [C, A], f32r, isOutput=False)
    wq_ext = nc.declare_dram_parameter("wq", [C, A], f32r, isOutput=False)
    bk2_ext = nc.declare_dram_parameter("bk2", [P, 1], f32, isOutput=False)
    bq2_ext = nc.declare_dram_parameter("bq2", [P, 1], f32, isOutput=False)
    wp_ext = nc.declare_dram_parameter("wp", [C, C], f32r, isOutput=False)
    bp_ext = nc.declare_dram_parameter("bp", [C, 1], f32, isOutput=False)
    w1_ext = nc.declare_dram_parameter("w1", [3, C, C], f32r, isOutput=False)
    b1_ext = nc.declare_dram_parameter("b1", [C, 1], f32, isOutput=False)
    w2_ext = nc.declare_dram_parameter("w2", [3, C, C], f32r, isOutput=False)
    b2_ext = nc.declare_dram_parameter("b2", [C, 1], f32, isOutput=False)
    g1_ext = nc.declare_dram_parameter("g1", [C, 1], f32, isOutput=False)
    be1_ext = nc.declare_dram_parameter("be1", [C, 1], f32, isOutput=False)
    g2_ext = nc.declare_dram_parameter("g2", [C, 1], f32, isOutput=False)
    be2_ext = nc.declare_dram_parameter("be2", [C, 1], f32, isOutput=False)
    out_ext = nc.declare_dram_parameter("out", [BL, C, L], f32, isOutput=True)

    cc0_in = nc.dram_tensor("cc0_in", [1, 1], f32)
    cc0_out = nc.dram_tensor("cc0_out", [1, 1], f32, addr_space="Shared")
    cc1_in = nc.dram_tensor("cc1_in", [P, 2 * CT], f32)
    cc1_out = nc.dram_tensor("cc1_out", [P, 2 * CT], f32, addr_space="Shared")
    cc2_in = nc.dram_tensor("cc2_in", [P, 2 * CT], f32)
    cc2_out = nc.dram_tensor("cc2_out", [P, 2 * CT], f32, addr_space="Shared")

    rg = [list(range(NCORES))]

    with TileContext(nc) as tc:
        with tc.tile_pool(name="persist", bufs=1) as pers, \
             tc.tile_pool(name="small", bufs=1) as small, \
             tc.tile_pool(name="ostage", bufs=4) as ostage, \
             tc.tile_pool(name="psum", bufs=8, space="PSUM") as psum:

            x2_sb = pers.tile([P, BL, CT, L], f32)

            # warmup collective: absorbs the first-collective setup cost
            # (~50us) under the input DMA loads
            nc.gpsimd.collective_compute(
                "AllReduce", mybir.AluOpType.add, replica_groups=rg,
                ins=[cc0_in[:].opt()], outs=[cc0_out[:].opt()])

            # per-channel parameter vectors -> [P, CT] layout
            def load_vec(ext, tag):
                t = small.tile([P, CT], f32, tag=tag)
                for ct in range(CT):
                    nc.gpsimd.dma_start(out=t[:, ct : ct + 1],
                                        in_=ext[ct * P:(ct + 1) * P, 0:1])
                return t

            bp_sb = load_vec(bp_ext, "bp")
            b1_sb = load_vec(b1_ext, "b1")
            b2_sb = load_vec(b2_ext, "b2")
            g1_sb = load_vec(g1_ext, "g1")
            be1_sb = load_vec(be1_ext, "be1")
            g2_sb = load_vec(g2_ext, "g2")
            be2_sb = load_vec(be2_ext, "be2")
            bk2_sb = small.tile([P, 1], f32, tag="bk2")
            nc.gpsimd.dma_start(out=bk2_sb[:], in_=bk2_ext[:])
            bq2_sb = small.tile([P, 1], f32, tag="bq2")
            nc.gpsimd.dma_start(out=bq2_sb[:], in_=bq2_ext[:])

            ccin1_sb = small.tile([P, 2 * CT], f32, tag="ccin1")
            ccout1_sb = small.tile([P, 2 * CT], f32, tag="ccout1")
            ccin2_sb = small.tile([P, 2 * CT], f32, tag="ccin2")
            ccout2_sb = small.tile([P, 2 * CT], f32, tag="ccout2")
            scale1 = small.tile([P, CT], f32, tag="scale1")
            bias1 = small.tile([P, CT], f32, tag="bias1")
            scale2 = small.tile([P, CT], f32, tag="scale2")
            bias2 = small.tile([P, CT], f32, tag="bias2")
            eps_sb = small.tile([P, 1], f32, tag="eps")
            nc.vector.memset(eps_sb[:], EPS)

            # ---------------- Phase A: attention ----------------
            with tc.tile_pool(name="phA", bufs=1) as pa, \
                 tc.tile_pool(name="phAb", bufs=1) as pab:
                x_sb = pa.tile([P, BL, CT, L], f32r)
                wk_sb = pa.tile([P, CT, A], f32r)
                wq_sb = pa.tile([P, CT, A], f32r)
                wp_sb = pa.tile([P, CT, C], f32r)
                # small kq weights first so the first kq matmuls only wait on x
                for ct in range(CT):
                    nc.sync.dma_start(out=wk_sb[:, ct, :], in_=wk_ext[ct * P:(ct + 1) * P, :])
                    nc.sync.dma_start(out=wq_sb[:, ct, :], in_=wq_ext[ct * P:(ct + 1) * P, :])
                for b in range(BL):
                    for ct in range(CT):
                        nc.sync.dma_start(out=x_sb[:, b, ct, :],
                                          in_=x_ext[b, ct * P:(ct + 1) * P, :])
                for ct in range(CT):
                    nc.sync.dma_start(out=wp_sb[:, ct, :], in_=wp_ext[ct * P:(ct + 1) * P, :])

                keys_sb = pa.tile([P, BL, L], f32r)     # partitions 0:64 used
                queries_sb = pa.tile([P, BL, L], f32r)
                for dst, w_sb, bias_sb in ((keys_sb, wk_sb, bk2_sb), (queries_sb, wq_sb, bq2_sb)):
                    for b in range(BL):
                        for mc in range(MC):
                            ms = slice(mc * 512, (mc + 1) * 512)
                            kps = psum.tile([P, 512], f32, tag="ps")
                            for ct in range(CT):
                                nc.tensor.matmul(
                                    out=kps[0:A, :],
                                    lhsT=w_sb[:, ct, :],
                                    rhs=x_sb[:, b, ct, ms],
                                    start=(ct == 0), stop=(ct == CT - 1))
                            nc.vector.tensor_scalar_add(out=dst[0:A, b, ms],
                                                        in0=kps[0:A, :],
                                                        scalar1=bias_sb[0:A])

                for b in range(BL):
                    xT_sb = pab.tile([P, LT, C], f32, tag="xT")
                    for lc in range(LT):
                        nc.sync.dma_start(out=xT_sb[:, lc, :],
                                          in_=xT_ext[b, lc * P:(lc + 1) * P, :])

                    e_sb = pab.tile([P, LT, L], f32r, tag="e")
                    rsp = pab.tile([P, LT, MC], f32, tag="rsp")
                    for lc in range(LT):
                        for mc in range(MC):
                            sps = psum.tile([P, 512], f32, tag="ps")
                            nc.tensor.matmul(
                                out=sps[:],
                                lhsT=keys_sb[0:A, b, lc * P:(lc + 1) * P],
                                rhs=queries_sb[0:A, b, mc * 512:(mc + 1) * 512],
                                start=True, stop=True)
                            nc.scalar.activation(
                                out=e_sb[:, lc, mc * 512:(mc + 1) * 512],
                                in_=sps[:], func=AF.Exp, scale=SM_SCALE,
                                accum_out=rsp[:, lc, mc:mc + 1])
                    rs = pab.tile([P, LT], f32, tag="rs")
                    nc.vector.tensor_add(out=rs[:], in0=rsp[:, :, 0], in1=rsp[:, :, 1])
                    rcp = pab.tile([P, LT], f32, tag="rcp")
                    nc.vector.reciprocal(out=rcp[:], in_=rs[:])

                    # xTs[l, c] = xT[l, c] / rowsum[l]  (softmax denom folded in)
                    xTs = pab.tile([P, LT, C], f32r, tag="xTs")
                    for lc in range(LT):
                        nc.vector.tensor_scalar_mul(out=xTs[:, lc, :],
                                                    in0=xT_sb[:, lc, :],
                                                    scalar1=rcp[:, lc:lc + 1])

                    ao_sb = pab.tile([P, CT, L], f32r, tag="ao")
                    for cc in range(CT):
                        for mc in range(MC):
                            aps = psum.tile([P, 512], f32, tag="ps")
                            for lc in range(LT):
                                nc.tensor.matmul(
                                    out=aps[:],
                                    lhsT=xTs[:, lc, cc * P:(cc + 1) * P],
                                    rhs=e_sb[:, lc, mc * 512:(mc + 1) * 512],
                                    start=(lc == 0), stop=(lc == LT - 1))
                            nc.scalar.activation(out=ao_sb[:, cc, mc * 512:(mc + 1) * 512],
                                                 in_=aps[:], func=AF.Copy)

                    for oc in range(CT):
                        for mc in range(MC):
                            ms = slice(mc * 512, (mc + 1) * 512)
                            pps = psum.tile([P, 512], f32, tag="ps")
                            for ct in range(CT):
                                nc.tensor.matmul(
                                    out=pps[:],
                                    lhsT=wp_sb[:, ct, oc * P:(oc + 1) * P],
                                    rhs=ao_sb[:, ct, ms],
                                    start=(ct == 0), stop=(ct == CT - 1))
                            # x2 = proj + bp + x
                            nc.vector.scalar_tensor_tensor(
                                out=x2_sb[:, b, oc, ms], in0=pps[:],
                                scalar=bp_sb[:, oc:oc + 1],
                                in1=x_sb[:, b, oc, ms].bitcast(f32),
                                op0=ALU.add, op1=ALU.add)

            # ---------------- BN1 stats + AllReduce ----------------
            def bn_partial_stats(src_sb, ccin_sb):
                for ct in range(CT):
                    st = small.tile([P, 2 * BL, 6], f32, tag="bnst")
                    i = 0
                    for b in range(BL):
                        for hc in range(MC):
                            nc.vector.bn_stats(out=st[:, i, :],
                                               in_=src_sb[:, b, ct, hc * 512:(hc + 1) * 512])
                            i += 1
                    mv = small.tile([P, 2], f32, tag="bnmv")
                    nc.vector.bn_aggr(out=mv[:], in_=st[:])
                    nc.vector.tensor_copy(out=ccin_sb[:, 2 * ct:2 * ct + 1], in_=mv[:, 0:1])
                    # E[x^2]_local = mean^2 + var
                    nc.vector.scalar_tensor_tensor(
                        out=ccin_sb[:, 2 * ct + 1:2 * ct + 2], in0=mv[:, 0:1],
                        scalar=mv[:, 0:1], in1=mv[:, 1:2],
                        op0=ALU.mult, op1=ALU.add)

            def bn_post(ccout_sb, g_sb, be_sb, scale_t, bias_t, tag):
                mg = small.tile([P, CT], f32, tag=tag + "mg")
                ex2 = small.tile([P, CT], f32, tag=tag + "ex2")
                nc.vector.tensor_scalar_mul(out=mg[:], in0=ccout_sb[:, 0::2],
                                            scalar1=1.0 / NCORES)
                nc.vector.tensor_scalar_mul(out=ex2[:], in0=ccout_sb[:, 1::2],
                                            scalar1=1.0 / NCORES)
                nvar = small.tile([P, CT], f32, tag=tag + "nv")
                # nvar = mean^2 - E[x^2] = -var
                nc.vector.tensor_tensor(out=nvar[:], in0=mg[:], in1=mg[:], op=ALU.mult)
                nc.vector.tensor_tensor(out=nvar[:], in0=nvar[:], in1=ex2[:], op=ALU.subtract)
                sd = small.tile([P, CT], f32, tag=tag + "sd")
                nc.scalar.activation(out=sd[:], in_=nvar[:], func=AF.Sqrt,
                                     scale=-1.0, bias=eps_sb[:])
                rstd = small.tile([P, CT], f32, tag=tag + "rstd")
                nc.vector.reciprocal(out=rstd[:], in_=sd[:])
                nc.vector.tensor_tensor(out=scale_t[:], in0=rstd[:], in1=g_sb[:], op=ALU.mult)
                tmp = small.tile([P, CT], f32, tag=tag + "tmp")
                nc.vector.tensor_tensor(out=tmp[:], in0=mg[:], in1=scale_t[:], op=ALU.mult)
                nc.vector.tensor_tensor(out=bias_t[:], in0=be_sb[:], in1=tmp[:], op=ALU.subtract)

            bn_partial_stats(x2_sb, ccin1_sb)

            with tc.tile_pool(name="phB", bufs=1) as pb:
                w1_sb = pb.tile([P, 3 * CT, C], f32r)
                for k in range(3):
                    for ct in range(CT):
                        nc.sync.dma_start(out=w1_sb[:, k * CT + ct, :],
                                          in_=w1_ext[k, ct * P:(ct + 1) * P, :])

                nc.gpsimd.dma_start(out=cc1_in[:], in_=ccin1_sb[:])
                nc.gpsimd.collective_compute(
                    "AllReduce", mybir.AluOpType.add, replica_groups=rg,
                    ins=[cc1_in[:].opt()], outs=[cc1_out[:].opt()])
                nc.gpsimd.dma_start(out=ccout1_sb[:], in_=cc1_out[:])
                bn_post(ccout1_sb, g1_sb, be1_sb, scale1, bias1, "p1")

                # h = relu(bn1(x2)), stored padded: h[:, b, ct, 1+l], zeros at 0 and L+1
                h_sb = pb.tile([P, BL, CT, L + 2], f32r, tag="hpad")
                # f32r memset is rejected by the ISA; write the pad zeros via ACT
                zsrc = eps_sb[:, 0:1].to_broadcast((P, BL, CT))
                nc.scalar.activation(out=h_sb[:, :, :, 0], in_=zsrc,
                                     func=AF.Copy, scale=0.0)
                nc.scalar.activation(out=h_sb[:, :, :, L + 1], in_=zsrc,
                                     func=AF.Copy, scale=0.0)
                for b in range(BL):
                    for ct in range(CT):
                        nc.scalar.activation(out=h_sb[:, b, ct, 1:L + 1],
                                             in_=x2_sb[:, b, ct, :], func=AF.Relu,
                                             scale=scale1[:, ct:ct + 1],
                                             bias=bias1[:, ct:ct + 1])

                w2_sb = pb.tile([P, 3 * CT, C], f32r)
                for k in range(3):
                    for ct in range(CT):
                        nc.sync.dma_start(out=w2_sb[:, k * CT + ct, :],
                                          in_=w2_ext[k, ct * P:(ct + 1) * P, :])

                # conv1: h2[o, l] = sum_{ct,k} w1[k][i, o].T @ h[i, l+k-1] + b1
                h2_sb = pb.tile([P, BL, CT, L], f32)
                for oc in range(CT):
                    cps = [psum.tile([P, 512], f32, tag="ps", name=f"cps{_j}") for _j in range(2 * BL)]
                    for ct in range(CT):
                        for k in range(3):
                            w_ap = w1_sb[:, k * CT + ct, oc * P:(oc + 1) * P]
                            for b in range(BL):
                                for hc in range(MC):
                                    nc.tensor.matmul(
                                        out=cps[2 * b + hc][:], lhsT=w_ap,
                                        rhs=h_sb[:, b, ct, hc * 512 + k:hc * 512 + k + 512],
                                        start=(ct == 0 and k == 0),
                                        stop=(ct == CT - 1 and k == 2))
                    for b in range(BL):
                        for hc in range(MC):
                            nc.vector.tensor_scalar_add(
                                out=h2_sb[:, b, oc, hc * 512:(hc + 1) * 512],
                                in0=cps[2 * b + hc][:], scalar1=b1_sb[:, oc:oc + 1])

                # BN2 stats + AllReduce
                bn_partial_stats(h2_sb, ccin2_sb)
                nc.gpsimd.dma_start(out=cc2_in[:], in_=ccin2_sb[:])
                nc.gpsimd.collective_compute(
                    "AllReduce", mybir.AluOpType.add, replica_groups=rg,
                    ins=[cc2_in[:].opt()], outs=[cc2_out[:].opt()])
                nc.gpsimd.dma_start(out=ccout2_sb[:], in_=cc2_out[:])
                bn_post(ccout2_sb, g2_sb, be2_sb, scale2, bias2, "p2")

                # h3 = relu(bn2(h2)) overwrites h_sb in place (pad zeros kept)
                for b in range(BL):
                    for ct in range(CT):
                        nc.scalar.activation(out=h_sb[:, b, ct, 1:L + 1],
                                             in_=h2_sb[:, b, ct, :], func=AF.Relu,
                                             scale=scale2[:, ct:ct + 1],
                                             bias=bias2[:, ct:ct + 1])

                # conv2 + b2 + residual(x2) -> out
                for oc in range(CT):
                    cps = [psum.tile([P, 512], f32, tag="ps", name=f"cps{_j}") for _j in range(2 * BL)]
                    for ct in range(CT):
                        for k in range(3):
                            w_ap = w2_sb[:, k * CT + ct, oc * P:(oc + 1) * P]
                            for b in range(BL):
                                for hc in range(MC):
                                    nc.tensor.matmul(
                                        out=cps[2 * b + hc][:], lhsT=w_ap,
                                        rhs=h_sb[:, b, ct, hc * 512 + k:hc * 512 + k + 512],
                                        start=(ct == 0 and k == 0),
                                        stop=(ct == CT - 1 and k == 2))
                    for b in range(BL):
                        for hc in range(MC):
                            og = ostage.tile([P, 512], f32, tag="og")
                            nc.vector.scalar_tensor_tensor(
                                out=og[:], in0=cps[2 * b + hc][:],
                                scalar=b2_sb[:, oc:oc + 1],
                                in1=x2_sb[:, b, oc, hc * 512:(hc + 1) * 512],
                                op0=ALU.add, op1=ALU.add)
                            nc.sync.dma_start(
                                out=out_ext[b, oc * P:(oc + 1) * P, hc * 512:(hc + 1) * 512],
                                in_=og[:])

    nc.compile()
    return nc


def _get_nc():
    if "nc" not in _CACHE:
        _CACHE["nc"] = _build()
    return _CACHE["nc"]


def _prep_in_maps(inputs):
    f = np.float32
    x = np.ascontiguousarray(inputs["x"], dtype=f)
    shared = {
        "wk": np.ascontiguousarray(inputs["Wk"].T, dtype=f),
        "wq": np.ascontiguousarray(inputs["Wq"].T, dtype=f),
        "bk2": np.concatenate([inputs["bk"], inputs["bk"]]).reshape(P, 1).astype(f),
        "bq2": np.concatenate([inputs["bq"], inputs["bq"]]).reshape(P, 1).astype(f),
        "wp": np.ascontiguousarray(inputs["Wp"].T, dtype=f),
        "bp": np.asarray(inputs["bp"], dtype=f).reshape(C, 1),
        "w1": np.ascontiguousarray(np.transpose(inputs["W1"], (2, 1, 0)), dtype=f),
        "b1": np.asarray(inputs["b1"], dtype=f).reshape(C, 1),
        "w2": np.ascontiguousarray(np.transpose(inputs["W2"], (2, 1, 0)), dtype=f),
        "b2": np.asarray(inputs["b2"], dtype=f).reshape(C, 1),
        "g1": np.asarray(inputs["g1"], dtype=f).reshape(C, 1),
        "be1": np.asarray(inputs["be1"], dtype=f).reshape(C, 1),
        "g2": np.asarray(inputs["g2"], dtype=f).reshape(C, 1),
        "be2": np.asarray(inputs["be2"], dtype=f).reshape(C, 1),
    }
    in_maps = []
    for i in range(NCORES):
        xl = np.ascontiguousarray(x[i * BL:(i + 1) * BL])
        xTl = np.ascontiguousarray(np.transpose(xl, (0, 2, 1)))
        m = {"x": xl, "xT": xTl}
        m.update(shared)
        in_maps.append(m)
    return in_maps


def kernel(**inputs) -> np.ndarray:
    from concourse import bass_utils
    nc = _get_nc()
    in_maps = _prep_in_maps(inputs)
    res = bass_utils.run_bass_kernel_spmd(nc, in_maps, list(range(NCORES)))
    return np.concatenate([r["out"] for r in res.results], axis=0)


# revision 12
# speedup vs baseline: 1.1772x; 1.0471x over previous
"""Trainium2 Bass kernel for nn_AttnResBlock (B=16, C=512, A=64, L=1024).

Data-parallel over batch: 8 cores x 2 batches each. Weights replicated.
BatchNorm (training mode, stats over (B, L)) needs global batch stats ->
two tiny [128, 8] f32 AllReduces (local mean / E[x^2] per channel).

All matmuls run as float32r (TF32-like, 1 cycle/row vs 4 for fp32).
Layouts chosen so no on-chip transposes are needed:
  - x      [b, c, l]  (c on partitions)  : kq-matmul rhs, residual
  - xT     [b, l, c]  (host-transposed)  : attn-output lhsT
  - keys2/queries2 [a2, l] with batch 0 in partitions 0:64, batch 1 in
    64:128 -> scores for both batches via PE row/col tiling.
"""
import numpy as np

P = 128
B, C, A, L = 16, 512, 64, 1024
NCORES = 8
BL = B // NCORES          # local batches per core
CT = C // P               # 4 channel tiles
LT = L // P               # 8 length tiles
MC = L // 512             # 2 moving chunks
EPS = 1e-5
SM_SCALE = 2.0 / L        # softmax scale: scores/(L/2)

_CACHE = {}


def _build():
    import concourse.bass as bass
    import concourse.mybir as mybir
    from concourse import bacc
    from concourse.tile import TileContext

    f32 = mybir.dt.float32
    f32r = mybir.dt.float32r
    bf16 = mybir.dt.bfloat16
    AF = mybir.ActivationFunctionType
    ALU = mybir.AluOpType

    nc = bacc.Bacc(num_devices=NCORES)

    x_ext = nc.declare_dram_parameter("x", [BL, C, L], f32r, isOutput=False)
    xT_ext = nc.declare_dram_parameter("xT", [BL, L, C], f32, isOutput=False)
    wk_ext = nc.declare_dram_parameter("wk", [C, A], f32r, isOutput=False)
    wq_ext = nc.declare_dram_parameter("wq", [C, A], f32r, isOutput=False)
    bk2_ext = nc.declare_dram_parameter("bk2", [P, 1], f32, isOutput=False)
    bq2_ext = nc.declare_dram_parameter("bq2", [P, 1], f32, isOutput=False)
    wp_ext = nc.declare_dram_parameter("wp", [C, C], bf16, isOutput=False)
    bp_ext = nc.declare_dram_parameter("bp", [C, 1], f32, isOutput=False)
    w1_ext = nc.declare_dram_parameter("w1", [3, C, C], bf16, isOutput=False)
    b1_ext = nc.declare_dram_parameter("b1", [C, 1], f32, isOutput=False)
    w2_ext = nc.declare_dram_parameter("w2", [3, C, C], bf16, isOutput=False)
    b2_ext = nc.declare_dram_parameter("b2", [C, 1], f32, isOutput=False)
    g1_ext = nc.declare_dram_parameter("g1", [C, 1], f32, isOutput=False)
    be1_ext = nc.declare_dram_parameter("be1", [C, 1], f32, isOutput=False)
    g2_ext = nc.declare_dram_parameter("g2", [C, 1], f32, isOutput=False)
    be2_ext = nc.declare_dram_parameter("be2", [C, 1], f32, isOutput=False)
    out_ext = nc.declare_dram_parameter("out", [BL, C, L], f32, isOutput=True)

    cc0_in = nc.dram_tensor("cc0_in", [1, 1], f32)
    cc0_out = nc.dram_tensor("cc0_out", [1, 1], f32, addr_space="Shared")
    cc1_in = nc.dram_tensor("cc1_in", [P, 2 * CT], f32)
    cc1_out = nc.dram_tensor("cc1_out", [P, 2 * CT], f32, addr_space="Shared")
    cc2_in = nc.dram_tensor("cc2_in", [P, 2 * CT], f32)
    cc2_out = nc.dram_tensor("cc2_out", [P, 2 * CT], f32, addr_space="Shared")

    rg = [list(range(NCORES))]

    with TileContext(nc) as tc:
        with tc.tile_pool(name="persist", bufs=1) as pers, \
             tc.tile_pool(name="small", bufs=1) as small, \
             tc.tile_pool(name="ostage", bufs=4) as ostage, \
             tc.tile_pool(name="psum", bufs=8, space="PSUM") as psum:

            x2_sb = pers.tile([P, BL, CT, L], f32)

            # warmup collective: absorbs the first-collective setup cost
            # (~50us) under the input DMA loads
            nc.gpsimd.collective_compute(
                "AllReduce", mybir.AluOpType.add, replica_groups=rg,
                ins=[cc0_in[:].opt()], outs=[cc0_out[:].opt()])

            # per-channel parameter vectors -> [P, CT] layout
            def load_vec(ext, tag):
                t = small.tile([P, CT], f32, tag=tag)
                for ct in range(CT):
                    nc.gpsimd.dma_start(out=t[:, ct : ct + 1],
                                        in_=ext[ct * P:(ct + 1) * P, 0:1])
                return t

            bp_sb = load_vec(bp_ext, "bp")
            b1_sb = load_vec(b1_ext, "b1")
            b2_sb = load_vec(b2_ext, "b2")
            g1_sb = load_vec(g1_ext, "g1")
            be1_sb = load_vec(be1_ext, "be1")
            g2_sb = load_vec(g2_ext, "g2")
            be2_sb = load_vec(be2_ext, "be2")
            bk2_sb = small.tile([P, 1], f32, tag="bk2")
            nc.gpsimd.dma_start(out=bk2_sb[:], in_=bk2_ext[:])
            bq2_sb = small.tile([P, 1], f32, tag="bq2")
            nc.gpsimd.dma_start(out=bq2_sb[:], in_=bq2_ext[:])

            ccin1_sb = small.tile([P, 2 * CT], f32, tag="ccin1")
            ccout1_sb = small.tile([P, 2 * CT], f32, tag="ccout1")
            ccin2_sb = small.tile([P, 2 * CT], f32, tag="ccin2")
            ccout2_sb = small.tile([P, 2 * CT], f32, tag="ccout2")
            scale1 = small.tile([P, CT], f32, tag="scale1")
            bias1 = small.tile([P, CT], f32, tag="bias1")
            scale2 = small.tile([P, CT], f32, tag="scale2")
            bias2 = small.tile([P, CT], f32, tag="bias2")
            eps_sb = small.tile([P, 1], f32, tag="eps")
            nc.vector.memset(eps_sb[:], EPS)

            # ---------------- Phase A: attention ----------------
            with tc.tile_pool(name="phA", bufs=1) as pa, \
                 tc.tile_pool(name="phAb", bufs=1) as pab:
                x_sb = pa.tile([P, BL, CT, L], f32r)
                wk_sb = pa.tile([P, CT, A], f32r)
                wq_sb = pa.tile([P, CT, A], f32r)
                wp_sb = pa.tile([P, CT, C], bf16)
                # small kq weights first so the first kq matmuls only wait on x
                for ct in range(CT):
                    nc.sync.dma_start(out=wk_sb[:, ct, :], in_=wk_ext[ct * P:(ct + 1) * P, :])
                    nc.sync.dma_start(out=wq_sb[:, ct, :], in_=wq_ext[ct * P:(ct + 1) * P, :])
                for b in range(BL):
                    for ct in range(CT):
                        nc.sync.dma_start(out=x_sb[:, b, ct, :],
                                          in_=x_ext[b, ct * P:(ct + 1) * P, :])
                for ct in range(CT):
                    nc.sync.dma_start(out=wp_sb[:, ct, :], in_=wp_ext[ct * P:(ct + 1) * P, :])

                keys_sb = pa.tile([P, BL, L], bf16)     # partitions 0:64 used
                queries_sb = pa.tile([P, BL, L], bf16)
                for dst, w_sb, bias_sb in ((keys_sb, wk_sb, bk2_sb), (queries_sb, wq_sb, bq2_sb)):
                    for b in range(BL):
                        for mc in range(MC):
                            ms = slice(mc * 512, (mc + 1) * 512)
                            kps = psum.tile([P, 512], f32, tag="ps")
                            for ct in range(CT):
                                nc.tensor.matmul(
                                    out=kps[0:A, :],
                                    lhsT=w_sb[:, ct, :],
                                    rhs=x_sb[:, b, ct, ms],
                                    start=(ct == 0), stop=(ct == CT - 1))
                            nc.vector.tensor_scalar_add(out=dst[0:A, b, ms],
                                                        in0=kps[0:A, :],
                                                        scalar1=bias_sb[0:A])

                for b in range(BL):
                    xT_sb = pab.tile([P, LT, C], f32, tag="xT")
                    for lc in range(LT):
                        nc.sync.dma_start(out=xT_sb[:, lc, :],
                                          in_=xT_ext[b, lc * P:(lc + 1) * P, :])

                    e_sb = pab.tile([P, LT, L], bf16, tag="e")
                    rsp = pab.tile([P, LT, MC], f32, tag="rsp")
                    for lc in range(LT):
                        for mc in range(MC):
                            sps = psum.tile([P, 512], f32, tag="ps")
                            nc.tensor.matmul(
                                out=sps[:],
                                lhsT=keys_sb[0:A, b, lc * P:(lc + 1) * P],
                                rhs=queries_sb[0:A, b, mc * 512:(mc + 1) * 512],
                                start=True, stop=True)
                            nc.scalar.activation(
                                out=e_sb[:, lc, mc * 512:(mc + 1) * 512],
                                in_=sps[:], func=AF.Exp, scale=SM_SCALE,
                                accum_out=rsp[:, lc, mc:mc + 1])
                    rs = pab.tile([P, LT], f32, tag="rs")
                    nc.vector.tensor_add(out=rs[:], in0=rsp[:, :, 0], in1=rsp[:, :, 1])
                    rcp = pab.tile([P, LT], f32, tag="rcp")
                    nc.vector.reciprocal(out=rcp[:], in_=rs[:])

                    # xTs[l, c] = xT[l, c] / rowsum[l]  (softmax denom folded in)
                    xTs = pab.tile([P, LT, C], bf16, tag="xTs")
                    for lc in range(LT):
                        nc.vector.tensor_scalar_mul(out=xTs[:, lc, :],
                                                    in0=xT_sb[:, lc, :],
                                                    scalar1=rcp[:, lc:lc + 1])

                    ao_sb = pab.tile([P, CT, L], bf16, tag="ao")
                    for cc in range(CT):
                        for mc in range(MC):
                            aps = psum.tile([P, 512], f32, tag="ps")
                            for lc in range(LT):
                                nc.tensor.matmul(
                                    out=aps[:],
                                    lhsT=xTs[:, lc, cc * P:(cc + 1) * P],
                                    rhs=e_sb[:, lc, mc * 512:(mc + 1) * 512],
                                    start=(lc == 0), stop=(lc == LT - 1))
                            nc.scalar.activation(out=ao_sb[:, cc, mc * 512:(mc + 1) * 512],
                                                 in_=aps[:], func=AF.Copy)

                    for oc in range(CT):
                        for mc in range(MC):
                            ms = slice(mc * 512, (mc + 1) * 512)
                            pps = psum.tile([P, 512], f32, tag="ps")
                            for ct in range(CT):
                                nc.tensor.matmul(
                                    out=pps[:],
                                    lhsT=wp_sb[:, ct, oc * P:(oc + 1) * P],
                                    rhs=ao_sb[:, ct, ms],
                                    start=(ct == 0), stop=(ct == CT - 1))
                            # x2 = proj + bp + x
                            nc.vector.scalar_tensor_tensor(
                                out=x2_sb[:, b, oc, ms], in0=pps[:],
                                scalar=bp_sb[:, oc:oc + 1],
                                in1=x_sb[:, b, oc, ms].bitcast(f32),
                                op0=ALU.add, op1=ALU.add)

            # ---------------- BN1 stats + AllReduce ----------------
            def bn_partial_stats(src_sb, ccin_sb):
                for ct in range(CT):
                    st = small.tile([P, 2 * BL, 6], f32, tag="bnst")
                    i = 0
                    for b in range(BL):
                        for hc in range(MC):
                            nc.vector.bn_stats(out=st[:, i, :],
                                               in_=src_sb[:, b, ct, hc * 512:(hc + 1) * 512])
                            i += 1
                    mv = small.tile([P, 2], f32, tag="bnmv")
                    nc.vector.bn_aggr(out=mv[:], in_=st[:])
                    nc.vector.tensor_copy(out=ccin_sb[:, 2 * ct:2 * ct + 1], in_=mv[:, 0:1])
                    # E[x^2]_local = mean^2 + var
                    nc.vector.scalar_tensor_tensor(
                        out=ccin_sb[:, 2 * ct + 1:2 * ct + 2], in0=mv[:, 0:1],
                        scalar=mv[:, 0:1], in1=mv[:, 1:2],
                        op0=ALU.mult, op1=ALU.add)

            def bn_post(ccout_sb, g_sb, be_sb, scale_t, bias_t, tag):
                mg = small.tile([P, CT], f32, tag=tag + "mg")
                ex2 = small.tile([P, CT], f32, tag=tag + "ex2")
                nc.vector.tensor_scalar_mul(out=mg[:], in0=ccout_sb[:, 0::2],
                                            scalar1=1.0 / NCORES)
                nc.vector.tensor_scalar_mul(out=ex2[:], in0=ccout_sb[:, 1::2],
                                            scalar1=1.0 / NCORES)
                nvar = small.tile([P, CT], f32, tag=tag + "nv")
                # nvar = mean^2 - E[x^2] = -var
                nc.vector.tensor_tensor(out=nvar[:], in0=mg[:], in1=mg[:], op=ALU.mult)
                nc.vector.tensor_tensor(out=nvar[:], in0=nvar[:], in1=ex2[:], op=ALU.subtract)
                sd = small.tile([P, CT], f32, tag=tag + "sd")
                nc.scalar.activation(out=sd[:], in_=nvar[:], func=AF.Sqrt,
                                     scale=-1.0, bias=eps_sb[:])
                rstd = small.tile([P, CT], f32, tag=tag + "rstd")
                nc.vector.reciprocal(out=rstd[:], in_=sd[:])
                nc.vector.tensor_tensor(out=scale_t[:], in0=rstd[:], in1=g_sb[:], op=ALU.mult)
                tmp = small.tile([P, CT], f32, tag=tag + "tmp")
                nc.vector.tensor_tensor(out=tmp[:], in0=mg[:], in1=scale_t[:], op=ALU.mult)
                nc.vector.tensor_tensor(out=bias_t[:], in0=be_sb[:], in1=tmp[:], op=ALU.subtract)

            bn_partial_stats(x2_sb, ccin1_sb)

            with tc.tile_pool(name="phB", bufs=1) as pb:
                w1_sb = pb.tile([P, 3 * CT, C], bf16)
                for k in range(3):
                    for ct in range(CT):
                        nc.sync.dma_start(out=w1_sb[:, k * CT + ct, :],
                                          in_=w1_ext[k, ct * P:(ct + 1) * P, :])

                nc.gpsimd.dma_start(out=cc1_in[:], in_=ccin1_sb[:])
                nc.gpsimd.collective_compute(
                    "AllReduce", mybir.AluOpType.add, replica_groups=rg,
                    ins=[cc1_in[:].opt()], outs=[cc1_out[:].opt()])
                nc.gpsimd.dma_start(out=ccout1_sb[:], in_=cc1_out[:])
                bn_post(ccout1_sb, g1_sb, be1_sb, scale1, bias1, "p1")

                # h = relu(bn1(x2)), stored padded: h[:, b, ct, 1+l], zeros at 0 and L+1
                h_sb = pb.tile([P, BL, CT, L + 2], bf16, tag="hpad")
                # f32r memset is rejected by the ISA; write the pad zeros via ACT
                zsrc = eps_sb[:, 0:1].to_broadcast((P, BL, CT))
                nc.scalar.activation(out=h_sb[:, :, :, 0], in_=zsrc,
                                     func=AF.Copy, scale=0.0)
                nc.scalar.activation(out=h_sb[:, :, :, L + 1], in_=zsrc,
                                     func=AF.Copy, scale=0.0)
                for b in range(BL):
                    for ct in range(CT):
                        nc.scalar.activation(out=h_sb[:, b, ct, 1:L + 1],
                                             in_=x2_sb[:, b, ct, :], func=AF.Relu,
                                             scale=scale1[:, ct:ct + 1],
                                             bias=bias1[:, ct:ct + 1])

                w2_sb = pb.tile([P, 3 * CT, C], bf16)
                for k in range(3):
                    for ct in range(CT):
                        nc.sync.dma_start(out=w2_sb[:, k * CT + ct, :],
                                          in_=w2_ext[k, ct * P:(ct + 1) * P, :])

                # conv1: h2[o, l] = sum_{ct,k} w1[k][i, o].T @ h[i, l+k-1] + b1
                h2_sb = pb.tile([P, BL, CT, L], f32)
                for oc in range(CT):
                    cps = [psum.tile([P, 512], f32, tag="ps", name=f"cps{_j}") for _j in range(2 * BL)]
                    for ct in range(CT):
                        for k in range(3):
                            w_ap = w1_sb[:, k * CT + ct, oc * P:(oc + 1) * P]
                            for b in range(BL):
                                for hc in range(MC):
                                    nc.tensor.matmul(
                                        out=cps[2 * b + hc][:], lhsT=w_ap,
                                        rhs=h_sb[:, b, ct, hc * 512 + k:hc * 512 + k + 512],
                                        start=(ct == 0 and k == 0),
                                        stop=(ct == CT - 1 and k == 2))
                    for b in range(BL):
                        for hc in range(MC):
                            nc.vector.tensor_scalar_add(
                                out=h2_sb[:, b, oc, hc * 512:(hc + 1) * 512],
                                in0=cps[2 * b + hc][:], scalar1=b1_sb[:, oc:oc + 1])

                # BN2 stats + AllReduce
                bn_partial_stats(h2_sb, ccin2_sb)
                nc.gpsimd.dma_start(out=cc2_in[:], in_=ccin2_sb[:])
                nc.gpsimd.collective_compute(
                    "AllReduce", mybir.AluOpType.add, replica_groups=rg,
                    ins=[cc2_in[:].opt()], outs=[cc2_out[:].opt()])
                nc.gpsimd.dma_start(out=ccout2_sb[:], in_=cc2_out[:])
                bn_post(ccout2_sb, g2_sb, be2_sb, scale2, bias2, "p2")

                # h3 = relu(bn2(h2)) overwrites h_sb in place (pad zeros kept)
                for b in range(BL):
                    for ct in range(CT):
                        nc.scalar.activation(out=h_sb[:, b, ct, 1:L + 1],
                                             in_=h2_sb[:, b, ct, :], func=AF.Relu,
                                             scale=scale2[:, ct:ct + 1],
                                             bias=bias2[:, ct:ct + 1])

                # conv2 + b2 + residual(x2) -> out
                for oc in range(CT):
                    cps = [psum.tile([P, 512], f32, tag="ps", name=f"cps{_j}") for _j in range(2 * BL)]
                    for ct in range(CT):
                        for k in range(3):
                            w_ap = w2_sb[:, k * CT + ct, oc * P:(oc + 1) * P]
                            for b in range(BL):
                                for hc in range(MC):
                                    nc.tensor.matmul(
                                        out=cps[2 * b + hc][:], lhsT=w_ap,
                                        rhs=h_sb[:, b, ct, hc * 512 + k:hc * 512 + k + 512],
                                        start=(ct == 0 and k == 0),
                                        stop=(ct == CT - 1 and k == 2))
                    for b in range(BL):
                        for hc in range(MC):
                            og = ostage.tile([P, 512], f32, tag="og")
                            nc.vector.scalar_tensor_tensor(
                                out=og[:], in0=cps[2 * b + hc][:],
                                scalar=b2_sb[:, oc:oc + 1],
                                in1=x2_sb[:, b, oc, hc * 512:(hc + 1) * 512],
                                op0=ALU.add, op1=ALU.add)
                            nc.sync.dma_start(
                                out=out_ext[b, oc * P:(oc + 1) * P, hc * 512:(hc + 1) * 512],
                                in_=og[:])

    nc.compile()
    return nc


def _get_nc():
    if "nc" not in _CACHE:
        _CACHE["nc"] = _build()
    return _CACHE["nc"]


def _prep_in_maps(inputs):
    import ml_dtypes
    f = np.float32
    bf = ml_dtypes.bfloat16
    x = np.ascontiguousarray(inputs["x"], dtype=f)
    shared = {
        "wk": np.ascontiguousarray(inputs["Wk"].T, dtype=f),
        "wq": np.ascontiguousarray(inputs["Wq"].T, dtype=f),
        "bk2": np.concatenate([inputs["bk"], inputs["bk"]]).reshape(P, 1).astype(f),
        "bq2": np.concatenate([inputs["bq"], inputs["bq"]]).reshape(P, 1).astype(f),
        "wp": np.ascontiguousarray(inputs["Wp"].T).astype(bf),
        "bp": np.asarray(inputs["bp"], dtype=f).reshape(C, 1),
        "w1": np.ascontiguousarray(np.transpose(inputs["W1"], (2, 1, 0))).astype(bf),
        "b1": np.asarray(inputs["b1"], dtype=f).reshape(C, 1),
        "w2": np.ascontiguousarray(np.transpose(inputs["W2"], (2, 1, 0))).astype(bf),
        "b2": np.asarray(inputs["b2"], dtype=f).reshape(C, 1),
        "g1": np.asarray(inputs["g1"], dtype=f).reshape(C, 1),
        "be1": np.asarray(inputs["be1"], dtype=f).reshape(C, 1),
        "g2": np.asarray(inputs["g2"], dtype=f).reshape(C, 1),
        "be2": np.asarray(inputs["be2"], dtype=f).reshape(C, 1),
    }
    in_maps = []
    for i in range(NCORES):
        xl = np.ascontiguousarray(x[i * BL:(i + 1) * BL])
        xTl = np.ascontiguousarray(np.transpose(xl, (0, 2, 1)))
        m = {"x": xl, "xT": xTl}
        m.update(shared)
        in_maps.append(m)
    return in_maps


def kernel(**inputs) -> np.ndarray:
    from concourse import bass_utils
    nc = _get_nc()
    in_maps = _prep_in_maps(inputs)
    res = bass_utils.run_bass_kernel_spmd(nc, in_maps, list(range(NCORES)))
    return np.concatenate([r["out"] for r in res.results], axis=0)


# revision 13
# speedup vs baseline: 1.2328x; 1.0472x over previous
"""Trainium2 Bass kernel for nn_AttnResBlock (B=16, C=512, A=64, L=1024).

Data-parallel over batch: 8 cores x 2 batches each. Weights replicated.
BatchNorm (training mode, stats over (B, L)) needs global batch stats ->
two tiny [128, 8] f32 AllReduces (local mean / E[x^2] per channel).

All matmuls run as float32r (TF32-like, 1 cycle/row vs 4 for fp32).
Layouts chosen so no on-chip transposes are needed:
  - x      [b, c, l]  (c on partitions)  : kq-matmul rhs, residual
  - xT     [b, l, c]  (host-transposed)  : attn-output lhsT
  - keys2/queries2 [a2, l] with batch 0 in partitions 0:64, batch 1 in
    64:128 -> scores for both batches via PE row/col tiling.
"""
import numpy as np

P = 128
B, C, A, L = 16, 512, 64, 1024
NCORES = 8
BL = B // NCORES          # local batches per core
CT = C // P               # 4 channel tiles
LT = L // P               # 8 length tiles
MC = L // 512             # 2 moving chunks
EPS = 1e-5
SM_SCALE = 2.0 / L        # softmax scale: scores/(L/2)

_CACHE = {}


def _build():
    import concourse.bass as bass
    import concourse.mybir as mybir
    from concourse import bacc
    from concourse.tile import TileContext

    f32 = mybir.dt.float32
    f32r = mybir.dt.float32r
    bf16 = mybir.dt.bfloat16
    AF = mybir.ActivationFunctionType
    ALU = mybir.AluOpType

    nc = bacc.Bacc(num_devices=NCORES)

    x_ext = nc.declare_dram_parameter("x", [BL, C, L], f32r, isOutput=False)
    xT_ext = nc.declare_dram_parameter("xT", [BL, L, C], f32, isOutput=False)
    wk_ext = nc.declare_dram_parameter("wk", [C, A], f32r, isOutput=False)
    wq_ext = nc.declare_dram_parameter("wq", [C, A], f32r, isOutput=False)
    bk2_ext = nc.declare_dram_parameter("bk2", [P, 1], f32, isOutput=False)
    bq2_ext = nc.declare_dram_parameter("bq2", [P, 1], f32, isOutput=False)
    wp_ext = nc.declare_dram_parameter("wp", [C, C], bf16, isOutput=False)
    bp_ext = nc.declare_dram_parameter("bp", [C, 1], f32, isOutput=False)
    w1_ext = nc.declare_dram_parameter("w1", [3, C, C], bf16, isOutput=False)
    b1_ext = nc.declare_dram_parameter("b1", [C, 1], f32, isOutput=False)
    w2_ext = nc.declare_dram_parameter("w2", [3, C, C], bf16, isOutput=False)
    b2_ext = nc.declare_dram_parameter("b2", [C, 1], f32, isOutput=False)
    g1_ext = nc.declare_dram_parameter("g1", [C, 1], f32, isOutput=False)
    be1_ext = nc.declare_dram_parameter("be1", [C, 1], f32, isOutput=False)
    g2_ext = nc.declare_dram_parameter("g2", [C, 1], f32, isOutput=False)
    be2_ext = nc.declare_dram_parameter("be2", [C, 1], f32, isOutput=False)
    out_ext = nc.declare_dram_parameter("out", [BL, C, L], f32, isOutput=True)

    cc0_in = nc.dram_tensor("cc0_in", [1, 1], f32)
    cc0_out = nc.dram_tensor("cc0_out", [1, 1], f32, addr_space="Shared")
    cc1_in = nc.dram_tensor("cc1_in", [P, 2 * CT], f32)
    cc1_out = nc.dram_tensor("cc1_out", [P, 2 * CT], f32, addr_space="Shared")
    cc2_in = nc.dram_tensor("cc2_in", [P, 2 * CT], f32)
    cc2_out = nc.dram_tensor("cc2_out", [P, 2 * CT], f32, addr_space="Shared")

    rg = [list(range(NCORES))]

    with TileContext(nc) as tc:
        with tc.tile_pool(name="persist", bufs=1) as pers, \
             tc.tile_pool(name="small", bufs=1) as small, \
             tc.tile_pool(name="ostage", bufs=4) as ostage, \
             tc.tile_pool(name="psum", bufs=8, space="PSUM") as psum:

            x2_sb = pers.tile([P, BL, CT, L], f32)

            # warmup collective: absorbs the first-collective setup cost
            # (~50us) under the input DMA loads
            nc.gpsimd.collective_compute(
                "AllReduce", mybir.AluOpType.add, replica_groups=rg,
                ins=[cc0_in[:].opt()], outs=[cc0_out[:].opt()])

            # per-channel parameter vectors -> [P, CT] layout
            def load_vec(ext, tag):
                t = small.tile([P, CT], f32, tag=tag)
                for ct in range(CT):
                    nc.gpsimd.dma_start(out=t[:, ct : ct + 1],
                                        in_=ext[ct * P:(ct + 1) * P, 0:1])
                return t

            bp_sb = load_vec(bp_ext, "bp")
            b1_sb = load_vec(b1_ext, "b1")
            b2_sb = load_vec(b2_ext, "b2")
            g1_sb = load_vec(g1_ext, "g1")
            be1_sb = load_vec(be1_ext, "be1")
            g2_sb = load_vec(g2_ext, "g2")
            be2_sb = load_vec(be2_ext, "be2")
            bk2_sb = small.tile([P, 1], f32, tag="bk2")
            nc.gpsimd.dma_start(out=bk2_sb[:], in_=bk2_ext[:])
            bq2_sb = small.tile([P, 1], f32, tag="bq2")
            nc.gpsimd.dma_start(out=bq2_sb[:], in_=bq2_ext[:])

            ccin1_sb = small.tile([P, 2 * CT], f32, tag="ccin1")
            ccout1_sb = small.tile([P, 2 * CT], f32, tag="ccout1")
            ccin2_sb = small.tile([P, 2 * CT], f32, tag="ccin2")
            ccout2_sb = small.tile([P, 2 * CT], f32, tag="ccout2")
            scale1 = small.tile([P, CT], f32, tag="scale1")
            bias1 = small.tile([P, CT], f32, tag="bias1")
            scale2 = small.tile([P, CT], f32, tag="scale2")
            bias2 = small.tile([P, CT], f32, tag="bias2")
            eps_sb = small.tile([P, 1], f32, tag="eps")
            nc.vector.memset(eps_sb[:], EPS)

            # ---------------- Phase A: attention ----------------
            with tc.tile_pool(name="phA", bufs=1) as pa, \
                 tc.tile_pool(name="phAb", bufs=2) as pab:
                x_sb = pa.tile([P, BL, CT, L], f32r)
                wk_sb = pa.tile([P, CT, A], f32r)
                wq_sb = pa.tile([P, CT, A], f32r)
                wp_sb = pa.tile([P, CT, C], bf16)
                # small kq weights first so the first kq matmuls only wait on x
                for ct in range(CT):
                    nc.sync.dma_start(out=wk_sb[:, ct, :], in_=wk_ext[ct * P:(ct + 1) * P, :])
                    nc.sync.dma_start(out=wq_sb[:, ct, :], in_=wq_ext[ct * P:(ct + 1) * P, :])
                for b in range(BL):
                    for ct in range(CT):
                        nc.sync.dma_start(out=x_sb[:, b, ct, :],
                                          in_=x_ext[b, ct * P:(ct + 1) * P, :])
                for ct in range(CT):
                    nc.sync.dma_start(out=wp_sb[:, ct, :], in_=wp_ext[ct * P:(ct + 1) * P, :])

                keys_sb = pa.tile([P, BL, L], bf16)     # partitions 0:64 used
                queries_sb = pa.tile([P, BL, L], bf16)
                for dst, w_sb, bias_sb in ((keys_sb, wk_sb, bk2_sb), (queries_sb, wq_sb, bq2_sb)):
                    for b in range(BL):
                        for mc in range(MC):
                            ms = slice(mc * 512, (mc + 1) * 512)
                            kps = psum.tile([P, 512], f32, tag="ps")
                            for ct in range(CT):
                                nc.tensor.matmul(
                                    out=kps[0:A, :],
                                    lhsT=w_sb[:, ct, :],
                                    rhs=x_sb[:, b, ct, ms],
                                    start=(ct == 0), stop=(ct == CT - 1))
                            nc.vector.tensor_scalar_add(out=dst[0:A, b, ms],
                                                        in0=kps[0:A, :],
                                                        scalar1=bias_sb[0:A])

                for b in range(BL):
                    xT_sb = pab.tile([P, LT, C], f32, tag="xT")
                    for lc in range(LT):
                        nc.sync.dma_start(out=xT_sb[:, lc, :],
                                          in_=xT_ext[b, lc * P:(lc + 1) * P, :])

                    e_sb = pab.tile([P, LT, L], bf16, tag="e")
                    rsp = pab.tile([P, LT, MC], f32, tag="rsp")
                    for lc in range(LT):
                        for mc in range(MC):
                            sps = psum.tile([P, 512], f32, tag="ps")
                            nc.tensor.matmul(
                                out=sps[:],
                                lhsT=keys_sb[0:A, b, lc * P:(lc + 1) * P],
                                rhs=queries_sb[0:A, b, mc * 512:(mc + 1) * 512],
                                start=True, stop=True)
                            nc.scalar.activation(
                                out=e_sb[:, lc, mc * 512:(mc + 1) * 512],
                                in_=sps[:], func=AF.Exp, scale=SM_SCALE,
                                accum_out=rsp[:, lc, mc:mc + 1])
                    rs = pab.tile([P, LT], f32, tag="rs")
                    nc.vector.tensor_add(out=rs[:], in0=rsp[:, :, 0], in1=rsp[:, :, 1])
                    rcp = pab.tile([P, LT], f32, tag="rcp")
                    nc.vector.reciprocal(out=rcp[:], in_=rs[:])

                    # xTs[l, c] = xT[l, c] / rowsum[l]  (softmax denom folded in)
                    xTs = pab.tile([P, LT, C], bf16, tag="xTs")
                    for lc in range(LT):
                        nc.vector.tensor_scalar_mul(out=xTs[:, lc, :],
                                                    in0=xT_sb[:, lc, :],
                                                    scalar1=rcp[:, lc:lc + 1])

                    ao_sb = pab.tile([P, CT, L], bf16, tag="ao")
                    for cc in range(CT):
                        for mc in range(MC):
                            aps = psum.tile([P, 512], f32, tag="ps")
                            for lc in range(LT):
                                nc.tensor.matmul(
                                    out=aps[:],
                                    lhsT=xTs[:, lc, cc * P:(cc + 1) * P],
                                    rhs=e_sb[:, lc, mc * 512:(mc + 1) * 512],
                                    start=(lc == 0), stop=(lc == LT - 1))
                            nc.scalar.activation(out=ao_sb[:, cc, mc * 512:(mc + 1) * 512],
                                                 in_=aps[:], func=AF.Copy)

                    for oc in range(CT):
                        for mc in range(MC):
                            ms = slice(mc * 512, (mc + 1) * 512)
                            pps = psum.tile([P, 512], f32, tag="ps")
                            for ct in range(CT):
                                nc.tensor.matmul(
                                    out=pps[:],
                                    lhsT=wp_sb[:, ct, oc * P:(oc + 1) * P],
                                    rhs=ao_sb[:, ct, ms],
                                    start=(ct == 0), stop=(ct == CT - 1))
                            # x2 = proj + bp + x
                            nc.vector.scalar_tensor_tensor(
                                out=x2_sb[:, b, oc, ms], in0=pps[:],
                                scalar=bp_sb[:, oc:oc + 1],
                                in1=x_sb[:, b, oc, ms].bitcast(f32),
                                op0=ALU.add, op1=ALU.add)

            # ---------------- BN1 stats + AllReduce ----------------
            def bn_partial_stats(src_sb, ccin_sb):
                for ct in range(CT):
                    st = small.tile([P, 2 * BL, 6], f32, tag="bnst")
                    i = 0
                    for b in range(BL):
                        for hc in range(MC):
                            nc.vector.bn_stats(out=st[:, i, :],
                                               in_=src_sb[:, b, ct, hc * 512:(hc + 1) * 512])
                            i += 1
                    mv = small.tile([P, 2], f32, tag="bnmv")
                    nc.vector.bn_aggr(out=mv[:], in_=st[:])
                    nc.vector.tensor_copy(out=ccin_sb[:, 2 * ct:2 * ct + 1], in_=mv[:, 0:1])
                    # E[x^2]_local = mean^2 + var
                    nc.vector.scalar_tensor_tensor(
                        out=ccin_sb[:, 2 * ct + 1:2 * ct + 2], in0=mv[:, 0:1],
                        scalar=mv[:, 0:1], in1=mv[:, 1:2],
                        op0=ALU.mult, op1=ALU.add)

            def bn_post(ccout_sb, g_sb, be_sb, scale_t, bias_t, tag):
                mg = small.tile([P, CT], f32, tag=tag + "mg")
                ex2 = small.tile([P, CT], f32, tag=tag + "ex2")
                nc.vector.tensor_scalar_mul(out=mg[:], in0=ccout_sb[:, 0::2],
                                            scalar1=1.0 / NCORES)
                nc.vector.tensor_scalar_mul(out=ex2[:], in0=ccout_sb[:, 1::2],
                                            scalar1=1.0 / NCORES)
                nvar = small.tile([P, CT], f32, tag=tag + "nv")
                # nvar = mean^2 - E[x^2] = -var
                nc.vector.tensor_tensor(out=nvar[:], in0=mg[:], in1=mg[:], op=ALU.mult)
                nc.vector.tensor_tensor(out=nvar[:], in0=nvar[:], in1=ex2[:], op=ALU.subtract)
                sd = small.tile([P, CT], f32, tag=tag + "sd")
                nc.scalar.activation(out=sd[:], in_=nvar[:], func=AF.Sqrt,
                                     scale=-1.0, bias=eps_sb[:])
                rstd = small.tile([P, CT], f32, tag=tag + "rstd")
                nc.vector.reciprocal(out=rstd[:], in_=sd[:])
                nc.vector.tensor_tensor(out=scale_t[:], in0=rstd[:], in1=g_sb[:], op=ALU.mult)
                tmp = small.tile([P, CT], f32, tag=tag + "tmp")
                nc.vector.tensor_tensor(out=tmp[:], in0=mg[:], in1=scale_t[:], op=ALU.mult)
                nc.vector.tensor_tensor(out=bias_t[:], in0=be_sb[:], in1=tmp[:], op=ALU.subtract)

            bn_partial_stats(x2_sb, ccin1_sb)

            with tc.tile_pool(name="phB", bufs=1) as pb:
                w1_sb = pb.tile([P, 3 * CT, C], bf16)
                for k in range(3):
                    for ct in range(CT):
                        nc.sync.dma_start(out=w1_sb[:, k * CT + ct, :],
                                          in_=w1_ext[k, ct * P:(ct + 1) * P, :])

                nc.gpsimd.dma_start(out=cc1_in[:], in_=ccin1_sb[:])
                nc.gpsimd.collective_compute(
                    "AllReduce", mybir.AluOpType.add, replica_groups=rg,
                    ins=[cc1_in[:].opt()], outs=[cc1_out[:].opt()])
                nc.gpsimd.dma_start(out=ccout1_sb[:], in_=cc1_out[:])
                bn_post(ccout1_sb, g1_sb, be1_sb, scale1, bias1, "p1")

                # h = relu(bn1(x2)), stored padded: h[:, b, ct, 1+l], zeros at 0 and L+1
                h_sb = pb.tile([P, BL, CT, L + 2], bf16, tag="hpad")
                # f32r memset is rejected by the ISA; write the pad zeros via ACT
                zsrc = eps_sb[:, 0:1].to_broadcast((P, BL, CT))
                nc.scalar.activation(out=h_sb[:, :, :, 0], in_=zsrc,
                                     func=AF.Copy, scale=0.0)
                nc.scalar.activation(out=h_sb[:, :, :, L + 1], in_=zsrc,
                                     func=AF.Copy, scale=0.0)
                for b in range(BL):
                    for ct in range(CT):
                        nc.scalar.activation(out=h_sb[:, b, ct, 1:L + 1],
                                             in_=x2_sb[:, b, ct, :], func=AF.Relu,
                                             scale=scale1[:, ct:ct + 1],
                                             bias=bias1[:, ct:ct + 1])

                w2_sb = pb.tile([P, 3 * CT, C], bf16)
                for k in range(3):
                    for ct in range(CT):
                        nc.sync.dma_start(out=w2_sb[:, k * CT + ct, :],
                                          in_=w2_ext[k, ct * P:(ct + 1) * P, :])

                # conv1: h2[o, l] = sum_{ct,k} w1[k][i, o].T @ h[i, l+k-1] + b1
                h2_sb = pb.tile([P, BL, CT, L], f32)
                for oc in range(CT):
                    cps = [psum.tile([P, 512], f32, tag="ps", name=f"cps{_j}") for _j in range(2 * BL)]
                    for ct in range(CT):
                        for k in range(3):
                            w_ap = w1_sb[:, k * CT + ct, oc * P:(oc + 1) * P]
                            for b in range(BL):
                                for hc in range(MC):
                                    nc.tensor.matmul(
                                        out=cps[2 * b + hc][:], lhsT=w_ap,
                                        rhs=h_sb[:, b, ct, hc * 512 + k:hc * 512 + k + 512],
                                        start=(ct == 0 and k == 0),
                                        stop=(ct == CT - 1 and k == 2))
                    for b in range(BL):
                        for hc in range(MC):
                            nc.vector.tensor_scalar_add(
                                out=h2_sb[:, b, oc, hc * 512:(hc + 1) * 512],
                                in0=cps[2 * b + hc][:], scalar1=b1_sb[:, oc:oc + 1])

                # BN2 stats + AllReduce
                bn_partial_stats(h2_sb, ccin2_sb)
                nc.gpsimd.dma_start(out=cc2_in[:], in_=ccin2_sb[:])
                nc.gpsimd.collective_compute(
                    "AllReduce", mybir.AluOpType.add, replica_groups=rg,
                    ins=[cc2_in[:].opt()], outs=[cc2_out[:].opt()])
                nc.gpsimd.dma_start(out=ccout2_sb[:], in_=cc2_out[:])
                bn_post(ccout2_sb, g2_sb, be2_sb, scale2, bias2, "p2")

                # h3 = relu(bn2(h2)) overwrites h_sb in place (pad zeros kept)
                for b in range(BL):
                    for ct in range(CT):
                        nc.scalar.activation(out=h_sb[:, b, ct, 1:L + 1],
                                             in_=h2_sb[:, b, ct, :], func=AF.Relu,
                                             scale=scale2[:, ct:ct + 1],
                                             bias=bias2[:, ct:ct + 1])

                # conv2 + b2 + residual(x2) -> out
                for oc in range(CT):
                    cps = [psum.tile([P, 512], f32, tag="ps", name=f"cps{_j}") for _j in range(2 * BL)]
                    for ct in range(CT):
                        for k in range(3):
                            w_ap = w2_sb[:, k * CT + ct, oc * P:(oc + 1) * P]
                            for b in range(BL):
                                for hc in range(MC):
                                    nc.tensor.matmul(
                                        out=cps[2 * b + hc][:], lhsT=w_ap,
                                        rhs=h_sb[:, b, ct, hc * 512 + k:hc * 512 + k + 512],
                                        start=(ct == 0 and k == 0),
                                        stop=(ct == CT - 1 and k == 2))
                    for b in range(BL):
                        for hc in range(MC):
                            og = ostage.tile([P, 512], f32, tag="og")
                            nc.vector.scalar_tensor_tensor(
                                out=og[:], in0=cps[2 * b + hc][:],
                                scalar=b2_sb[:, oc:oc + 1],
                                in1=x2_sb[:, b, oc, hc * 512:(hc + 1) * 512],
                                op0=ALU.add, op1=ALU.add)
                            nc.sync.dma_start(
                                out=out_ext[b, oc * P:(oc + 1) * P, hc * 512:(hc + 1) * 512],
                                in_=og[:])

    nc.compile()
    return nc


def _get_nc():
    if "nc" not in _CACHE:
        _CACHE["nc"] = _build()
    return _CACHE["nc"]


def _prep_in_maps(inputs):
    import ml_dtypes
    f = np.float32
    bf = ml_dtypes.bfloat16
    x = np.ascontiguousarray(inputs["x"], dtype=f)
    shared = {
        "wk": np.ascontiguousarray(inputs["Wk"].T, dtype=f),
        "wq": np.ascontiguousarray(inputs["Wq"].T, dtype=f),
        "bk2": np.concatenate([inputs["bk"], inputs["bk"]]).reshape(P, 1).astype(f),
        "bq2": np.concatenate([inputs["bq"], inputs["bq"]]).reshape(P, 1).astype(f),
        "wp": np.ascontiguousarray(inputs["Wp"].T).astype(bf),
        "bp": np.asarray(inputs["bp"], dtype=f).reshape(C, 1),
        "w1": np.ascontiguousarray(np.transpose(inputs["W1"], (2, 1, 0))).astype(bf),
        "b1": np.asarray(inputs["b1"], dtype=f).reshape(C, 1),
        "w2": np.ascontiguousarray(np.transpose(inputs["W2"], (2, 1, 0))).astype(bf),
        "b2": np.asarray(inputs["b2"], dtype=f).reshape(C, 1),
        "g1": np.asarray(inputs["g1"], dtype=f).reshape(C, 1),
        "be1": np.asarray(inputs["be1"], dtype=f).reshape(C, 1),
        "g2": np.asarray(inputs["g2"], dtype=f).reshape(C, 1),
        "be2": np.asarray(inputs["be2"], dtype=f).reshape(C, 1),
    }
    in_maps = []
    for i in range(NCORES):
        xl = np.ascontiguousarray(x[i * BL:(i + 1) * BL])
        xTl = np.ascontiguousarray(np.transpose(xl, (0, 2, 1)))
        m = {"x": xl, "xT": xTl}
        m.update(shared)
        in_maps.append(m)
    return in_maps


def kernel(**inputs) -> np.ndarray:
    from concourse import bass_utils
    nc = _get_nc()
    in_maps = _prep_in_maps(inputs)
    res = bass_utils.run_bass_kernel_spmd(nc, in_maps, list(range(NCORES)))
    return np.concatenate([r["out"] for r in res.results], axis=0)


# revision 20
# speedup vs baseline: 1.2692x; 1.0296x over previous
"""Trainium2 Bass kernel for nn_AttnResBlock (B=16, C=512, A=64, L=1024).

Data-parallel over batch: 8 cores x 2 batches each. Weights replicated.
BatchNorm (training mode, stats over (B, L)) needs global batch stats ->
two tiny [128, 8] f32 AllReduces (local mean / E[x^2] per channel).

All matmuls run as float32r (TF32-like, 1 cycle/row vs 4 for fp32).
Layouts chosen so no on-chip transposes are needed:
  - x      [b, c, l]  (c on partitions)  : kq-matmul rhs, residual
  - xT     [b, l, c]  (host-transposed)  : attn-output lhsT
  - keys2/queries2 [a2, l] with batch 0 in partitions 0:64, batch 1 in
    64:128 -> scores for both batches via PE row/col tiling.
"""
import numpy as np

P = 128
B, C, A, L = 16, 512, 64, 1024
NCORES = 8
BL = B // NCORES          # local batches per core
CT = C // P               # 4 channel tiles
LT = L // P               # 8 length tiles
MC = L // 512             # 2 moving chunks
EPS = 1e-5
SM_SCALE = 2.0 / L        # softmax scale: scores/(L/2)

_CACHE = {}


def _build():
    import concourse.bass as bass
    import concourse.mybir as mybir
    from concourse import bacc
    from concourse.tile import TileContext

    f32 = mybir.dt.float32
    f32r = mybir.dt.float32r
    bf16 = mybir.dt.bfloat16
    AF = mybir.ActivationFunctionType
    ALU = mybir.AluOpType

    nc = bacc.Bacc(num_devices=NCORES)

    x_ext = nc.declare_dram_parameter("x", [BL, C, L], f32r, isOutput=False)
    xT_ext = nc.declare_dram_parameter("xT", [BL, L, C], f32, isOutput=False)
    wk_ext = nc.declare_dram_parameter("wk", [C, A], f32r, isOutput=False)
    wq_ext = nc.declare_dram_parameter("wq", [C, A], f32r, isOutput=False)
    bk2_ext = nc.declare_dram_parameter("bk2", [P, 1], f32, isOutput=False)
    bq2_ext = nc.declare_dram_parameter("bq2", [P, 1], f32, isOutput=False)
    wp_ext = nc.declare_dram_parameter("wp", [C, C], bf16, isOutput=False)
    bp_ext = nc.declare_dram_parameter("bp", [C, 1], f32, isOutput=False)
    w1_ext = nc.declare_dram_parameter("w1", [3, C, C], bf16, isOutput=False)
    b1_ext = nc.declare_dram_parameter("b1", [C, 1], f32, isOutput=False)
    w2_ext = nc.declare_dram_parameter("w2", [3, C, C], bf16, isOutput=False)
    b2_ext = nc.declare_dram_parameter("b2", [C, 1], f32, isOutput=False)
    g1_ext = nc.declare_dram_parameter("g1", [C, 1], f32, isOutput=False)
    be1_ext = nc.declare_dram_parameter("be1", [C, 1], f32, isOutput=False)
    g2_ext = nc.declare_dram_parameter("g2", [C, 1], f32, isOutput=False)
    be2_ext = nc.declare_dram_parameter("be2", [C, 1], f32, isOutput=False)
    out_ext = nc.declare_dram_parameter("out", [BL, C, L], f32, isOutput=True)

    cc0_in = nc.dram_tensor("cc0_in", [1, 1], f32)
    cc0_out = nc.dram_tensor("cc0_out", [1, 1], f32, addr_space="Shared")
    cc1_in = nc.dram_tensor("cc1_in", [P, 2 * CT], f32)
    cc1_out = nc.dram_tensor("cc1_out", [P, 2 * CT], f32, addr_space="Shared")
    cc2_in = nc.dram_tensor("cc2_in", [P, 2 * CT], f32)
    cc2_out = nc.dram_tensor("cc2_out", [P, 2 * CT], f32, addr_space="Shared")

    rg = [list(range(NCORES))]

    with TileContext(nc) as tc:
        with tc.tile_pool(name="persist", bufs=1) as pers, \
             tc.tile_pool(name="small", bufs=1) as small, \
             tc.tile_pool(name="ostage", bufs=4) as ostage, \
             tc.tile_pool(name="psum", bufs=8, space="PSUM") as psum:

            x2_sb = pers.tile([P, BL, CT, L], f32)

            # warmup collective: absorbs the first-collective setup cost
            # (~50us) under the input DMA loads
            nc.gpsimd.collective_compute(
                "AllReduce", mybir.AluOpType.add, replica_groups=rg,
                ins=[cc0_in[:].opt()], outs=[cc0_out[:].opt()])

            # per-channel parameter vectors -> [P, CT] layout
            def load_vec(ext, tag):
                t = small.tile([P, CT], f32, tag=tag)
                for ct in range(CT):
                    nc.gpsimd.dma_start(out=t[:, ct : ct + 1],
                                        in_=ext[ct * P:(ct + 1) * P, 0:1])
                return t

            bp_sb = load_vec(bp_ext, "bp")
            b1_sb = load_vec(b1_ext, "b1")
            b2_sb = load_vec(b2_ext, "b2")
            g1_sb = load_vec(g1_ext, "g1")
            be1_sb = load_vec(be1_ext, "be1")
            g2_sb = load_vec(g2_ext, "g2")
            be2_sb = load_vec(be2_ext, "be2")
            bk2_sb = small.tile([P, 1], f32, tag="bk2")
            nc.gpsimd.dma_start(out=bk2_sb[:], in_=bk2_ext[:])
            bq2_sb = small.tile([P, 1], f32, tag="bq2")
            nc.gpsimd.dma_start(out=bq2_sb[:], in_=bq2_ext[:])

            ccin1_sb = small.tile([P, 2 * CT], f32, tag="ccin1")
            ccout1_sb = small.tile([P, 2 * CT], f32, tag="ccout1")
            ccin2_sb = small.tile([P, 2 * CT], f32, tag="ccin2")
            ccout2_sb = small.tile([P, 2 * CT], f32, tag="ccout2")
            # per-chunk stat accumulators: [P, ct, 2*b+hc] sums over 512-chunks
            m1a = small.tile([P, CT, 2 * BL], f32, tag="m1a")   # sum(x2)
            m2a = small.tile([P, CT, 2 * BL], f32, tag="m2a")   # sum(x2^2)
            n1a = small.tile([P, CT, 2 * BL], f32, tag="n1a")   # sum(h2)
            n2a = small.tile([P, CT, 2 * BL], f32, tag="n2a")   # sum(h2^2)
            scale1 = small.tile([P, CT], f32, tag="scale1")
            bias1 = small.tile([P, CT], f32, tag="bias1")
            scale2 = small.tile([P, CT], f32, tag="scale2")
            bias2 = small.tile([P, CT], f32, tag="bias2")
            eps_sb = small.tile([P, 1], f32, tag="eps")
            nc.vector.memset(eps_sb[:], EPS)

            # ---------------- Phase A: attention ----------------
            with tc.tile_pool(name="phA", bufs=1) as pa, \
                 tc.tile_pool(name="phAb", bufs=2) as pab:
                x_sb = pa.tile([P, BL, CT, L], f32r)
                wk_sb = pa.tile([P, CT, A], f32r)
                wq_sb = pa.tile([P, CT, A], f32r)
                wp_sb = pa.tile([P, CT, C], bf16)
                # small kq weights first so the first kq matmuls only wait on x
                for ct in range(CT):
                    nc.sync.dma_start(out=wk_sb[:, ct, :], in_=wk_ext[ct * P:(ct + 1) * P, :])
                    nc.sync.dma_start(out=wq_sb[:, ct, :], in_=wq_ext[ct * P:(ct + 1) * P, :])
                for b in range(BL):
                    for ct in range(CT):
                        nc.sync.dma_start(out=x_sb[:, b, ct, :],
                                          in_=x_ext[b, ct * P:(ct + 1) * P, :])
                for ct in range(CT):
                    nc.sync.dma_start(out=wp_sb[:, ct, :], in_=wp_ext[ct * P:(ct + 1) * P, :])

                keys_sb = pa.tile([P, BL, L], bf16)     # partitions 0:64 used
                queries_sb = pa.tile([P, BL, L], bf16)
                for dst, w_sb, bias_sb in ((keys_sb, wk_sb, bk2_sb), (queries_sb, wq_sb, bq2_sb)):
                    for b in range(BL):
                        for mc in range(MC):
                            ms = slice(mc * 512, (mc + 1) * 512)
                            kps = psum.tile([P, 512], f32, tag="ps")
                            for ct in range(CT):
                                nc.tensor.matmul(
                                    out=kps[0:A, :],
                                    lhsT=w_sb[:, ct, :],
                                    rhs=x_sb[:, b, ct, ms],
                                    start=(ct == 0), stop=(ct == CT - 1))
                            nc.vector.tensor_scalar_add(out=dst[0:A, b, ms],
                                                        in0=kps[0:A, :],
                                                        scalar1=bias_sb[0:A])

                for b in range(BL):
                    xT_sb = pab.tile([P, LT, C], f32, tag="xT")
                    for lc in range(LT):
                        nc.sync.dma_start(out=xT_sb[:, lc, :],
                                          in_=xT_ext[b, lc * P:(lc + 1) * P, :])

                    e_sb = pab.tile([P, LT, L], bf16, tag="e")
                    rsp = pab.tile([P, LT, MC], f32, tag="rsp")
                    rcp = pab.tile([P, LT], f32, tag="rcp")
                    xTs = pab.tile([P, LT, C], bf16, tag="xTs")
                    for lc in range(LT):
                        for mc in range(MC):
                            sps = psum.tile([P, 512], f32, tag="ps")
                            nc.tensor.matmul(
                                out=sps[:],
                                lhsT=keys_sb[0:A, b, lc * P:(lc + 1) * P],
                                rhs=queries_sb[0:A, b, mc * 512:(mc + 1) * 512],
                                start=True, stop=True)
                            nc.scalar.activation(
                                out=e_sb[:, lc, mc * 512:(mc + 1) * 512],
                                in_=sps[:], func=AF.Exp, scale=SM_SCALE,
                                accum_out=rsp[:, lc, mc:mc + 1])
                        # per-lc chain: attnout's lc-th matmul unblocks as soon
                        # as this lc's softmax denominator exists
                        nc.vector.scalar_tensor_tensor(
                            out=rcp[:, lc:lc + 1], in0=rsp[:, lc, 0:1],
                            scalar=1.0, in1=rsp[:, lc, 1:2],
                            op0=ALU.mult, op1=ALU.add)
                        nc.vector.reciprocal(out=rcp[:, lc:lc + 1],
                                             in_=rcp[:, lc:lc + 1])
                        # xTs[l, c] = xT[l, c] / rowsum[l] (softmax denom folded in)
                        nc.vector.tensor_scalar_mul(out=xTs[:, lc, :],
                                                    in0=xT_sb[:, lc, :],
                                                    scalar1=rcp[:, lc:lc + 1])

                    ao_sb = pab.tile([P, CT, L], bf16, tag="ao")
                    for cc in range(CT):
                        for mc in range(MC):
                            aps = psum.tile([P, 512], f32, tag="ps")
                            for lc in range(LT):
                                nc.tensor.matmul(
                                    out=aps[:],
                                    lhsT=xTs[:, lc, cc * P:(cc + 1) * P],
                                    rhs=e_sb[:, lc, mc * 512:(mc + 1) * 512],
                                    start=(lc == 0), stop=(lc == LT - 1))
                            nc.scalar.activation(out=ao_sb[:, cc, mc * 512:(mc + 1) * 512],
                                                 in_=aps[:], func=AF.Copy)

                    for oc in range(CT):
                        for mc in range(MC):
                            ms = slice(mc * 512, (mc + 1) * 512)
                            pps = psum.tile([P, 512], f32, tag="ps")
                            for ct in range(CT):
                                nc.tensor.matmul(
                                    out=pps[:],
                                    lhsT=wp_sb[:, ct, oc * P:(oc + 1) * P],
                                    rhs=ao_sb[:, ct, ms],
                                    start=(ct == 0), stop=(ct == CT - 1))
                            # x2 = proj + bp + x ; accum_out = per-chunk channel sums
                            nc.vector.scalar_tensor_tensor(
                                out=x2_sb[:, b, oc, ms], in0=pps[:],
                                scalar=bp_sb[:, oc:oc + 1],
                                in1=x_sb[:, b, oc, ms].bitcast(f32),
                                op0=ALU.add, op1=ALU.add,
                                accum_out=m1a[:, oc, 2 * b + mc:2 * b + mc + 1])
                            # sum(x2^2) for BN1 var (ACT square + accumulator)
                            sqs = ostage.tile([P, 512], f32, tag="sqs")
                            nc.scalar.activation(
                                out=sqs[:], in_=x2_sb[:, b, oc, ms], func=AF.Square,
                                accum_out=m2a[:, oc, 2 * b + mc:2 * b + mc + 1])

            # ---------------- BN1 stats + AllReduce ----------------
            def pack_stats(msum, sqsum, ccin_sb):
                # ccin layout: cols 0:CT = per-channel sum, CT:2CT = sum of squares
                nc.vector.tensor_reduce(out=ccin_sb[:, 0:CT], in_=msum[:],
                                        axis=mybir.AxisListType.X, op=ALU.add)
                nc.vector.tensor_reduce(out=ccin_sb[:, CT:2 * CT], in_=sqsum[:],
                                        axis=mybir.AxisListType.X, op=ALU.add)

            def bn_post(ccout_sb, g_sb, be_sb, scale_t, bias_t, tag):
                mg = small.tile([P, CT], f32, tag=tag + "mg")
                ex2 = small.tile([P, CT], f32, tag=tag + "ex2")
                nc.vector.tensor_scalar_mul(out=mg[:], in0=ccout_sb[:, 0:CT],
                                            scalar1=1.0 / (B * L))
                nc.vector.tensor_scalar_mul(out=ex2[:], in0=ccout_sb[:, CT:2 * CT],
                                            scalar1=1.0 / (B * L))
                nvar = small.tile([P, CT], f32, tag=tag + "nv")
                # nvar = mean^2 - E[x^2] = -var
                nc.vector.tensor_tensor(out=nvar[:], in0=mg[:], in1=mg[:], op=ALU.mult)
                nc.vector.tensor_tensor(out=nvar[:], in0=nvar[:], in1=ex2[:], op=ALU.subtract)
                sd = small.tile([P, CT], f32, tag=tag + "sd")
                nc.scalar.activation(out=sd[:], in_=nvar[:], func=AF.Sqrt,
                                     scale=-1.0, bias=eps_sb[:])
                rstd = small.tile([P, CT], f32, tag=tag + "rstd")
                nc.vector.reciprocal(out=rstd[:], in_=sd[:])
                nc.vector.tensor_tensor(out=scale_t[:], in0=rstd[:], in1=g_sb[:], op=ALU.mult)
                tmp = small.tile([P, CT], f32, tag=tag + "tmp")
                nc.vector.tensor_tensor(out=tmp[:], in0=mg[:], in1=scale_t[:], op=ALU.mult)
                nc.vector.tensor_tensor(out=bias_t[:], in0=be_sb[:], in1=tmp[:], op=ALU.subtract)

            pack_stats(m1a, m2a, ccin1_sb)

            with tc.tile_pool(name="phB", bufs=1) as pb:
                w1_sb = pb.tile([P, 3 * CT, C], bf16)
                for k in range(3):
                    for ct in range(CT):
                        nc.sync.dma_start(out=w1_sb[:, k * CT + ct, :],
                                          in_=w1_ext[k, ct * P:(ct + 1) * P, :])

                nc.gpsimd.dma_start(out=cc1_in[:], in_=ccin1_sb[:])
                nc.gpsimd.collective_compute(
                    "AllReduce", mybir.AluOpType.add, replica_groups=rg,
                    ins=[cc1_in[:].opt()], outs=[cc1_out[:].opt()])
                nc.gpsimd.dma_start(out=ccout1_sb[:], in_=cc1_out[:])
                bn_post(ccout1_sb, g1_sb, be1_sb, scale1, bias1, "p1")

                # h = relu(bn1(x2)), stored padded: h[:, b, ct, 1+l], zeros at 0 and L+1
                h_sb = pb.tile([P, BL, CT, L + 2], bf16, tag="hpad")
                # f32r memset is rejected by the ISA; write the pad zeros via ACT
                zsrc = eps_sb[:, 0:1].to_broadcast((P, BL, CT))
                nc.scalar.activation(out=h_sb[:, :, :, 0], in_=zsrc,
                                     func=AF.Copy, scale=0.0)
                nc.scalar.activation(out=h_sb[:, :, :, L + 1], in_=zsrc,
                                     func=AF.Copy, scale=0.0)
                for b in range(BL):
                    for ct in range(CT):
                        nc.scalar.activation(out=h_sb[:, b, ct, 1:L + 1],
                                             in_=x2_sb[:, b, ct, :], func=AF.Relu,
                                             scale=scale1[:, ct:ct + 1],
                                             bias=bias1[:, ct:ct + 1])

                w2_sb = pb.tile([P, 3 * CT, C], bf16)
                for k in range(3):
                    for ct in range(CT):
                        nc.sync.dma_start(out=w2_sb[:, k * CT + ct, :],
                                          in_=w2_ext[k, ct * P:(ct + 1) * P, :])

                # conv1: h2[o, l] = sum_{ct,k} w1[k][i, o].T @ h[i, l+k-1] + b1
                h2_sb = pb.tile([P, BL, CT, L], f32)
                for oc in range(CT):
                    cps = [psum.tile([P, 512], f32, tag="ps", name=f"cps{_j}") for _j in range(2 * BL)]
                    for ct in range(CT):
                        for k in range(3):
                            w_ap = w1_sb[:, k * CT + ct, oc * P:(oc + 1) * P]
                            for b in range(BL):
                                for hc in range(MC):
                                    nc.tensor.matmul(
                                        out=cps[2 * b + hc][:], lhsT=w_ap,
                                        rhs=h_sb[:, b, ct, hc * 512 + k:hc * 512 + k + 512],
                                        start=(ct == 0 and k == 0),
                                        stop=(ct == CT - 1 and k == 2))
                    for b in range(BL):
                        for hc in range(MC):
                            hs = slice(hc * 512, (hc + 1) * 512)
                            nc.vector.tensor_scalar(
                                out=h2_sb[:, b, oc, hs],
                                in0=cps[2 * b + hc][:], scalar1=b1_sb[:, oc:oc + 1],
                                scalar2=0.0, op0=ALU.add, op1=ALU.add,
                                accum_out=n1a[:, oc, 2 * b + hc:2 * b + hc + 1])
                            sqs = ostage.tile([P, 512], f32, tag="sqs")
                            nc.scalar.activation(
                                out=sqs[:], in_=h2_sb[:, b, oc, hs], func=AF.Square,
                                accum_out=n2a[:, oc, 2 * b + hc:2 * b + hc + 1])

                # BN2 stats + AllReduce
                pack_stats(n1a, n2a, ccin2_sb)
                nc.gpsimd.dma_start(out=cc2_in[:], in_=ccin2_sb[:])
                nc.gpsimd.collective_compute(
                    "AllReduce", mybir.AluOpType.add, replica_groups=rg,
                    ins=[cc2_in[:].opt()], outs=[cc2_out[:].opt()])
                nc.gpsimd.dma_start(out=ccout2_sb[:], in_=cc2_out[:])
                bn_post(ccout2_sb, g2_sb, be2_sb, scale2, bias2, "p2")

                # h3 = relu(bn2(h2)) overwrites h_sb in place (pad zeros kept)
                for b in range(BL):
                    for ct in range(CT):
                        nc.scalar.activation(out=h_sb[:, b, ct, 1:L + 1],
                                             in_=h2_sb[:, b, ct, :], func=AF.Relu,
                                             scale=scale2[:, ct:ct + 1],
                                             bias=bias2[:, ct:ct + 1])

                # conv2 + b2 + residual(x2) -> out
                for oc in range(CT):
                    cps = [psum.tile([P, 512], f32, tag="ps", name=f"cps{_j}") for _j in range(2 * BL)]
                    for ct in range(CT):
                        for k in range(3):
                            w_ap = w2_sb[:, k * CT + ct, oc * P:(oc + 1) * P]
                            for b in range(BL):
                                for hc in range(MC):
                                    nc.tensor.matmul(
                                        out=cps[2 * b + hc][:], lhsT=w_ap,
                                        rhs=h_sb[:, b, ct, hc * 512 + k:hc * 512 + k + 512],
                                        start=(ct == 0 and k == 0),
                                        stop=(ct == CT - 1 and k == 2))
                    for b in range(BL):
                        for hc in range(MC):
                            og = ostage.tile([P, 512], f32, tag="og")
                            nc.vector.scalar_tensor_tensor(
                                out=og[:], in0=cps[2 * b + hc][:],
                                scalar=b2_sb[:, oc:oc + 1],
                                in1=x2_sb[:, b, oc, hc * 512:(hc + 1) * 512],
                                op0=ALU.add, op1=ALU.add)
                            nc.sync.dma_start(
                                out=out_ext[b, oc * P:(oc + 1) * P, hc * 512:(hc + 1) * 512],
                                in_=og[:])

    nc.compile()
    return nc


def _get_nc():
    if "nc" not in _CACHE:
        _CACHE["nc"] = _build()
    return _CACHE["nc"]


def _prep_in_maps(inputs):
    import ml_dtypes
    f = np.float32
    bf = ml_dtypes.bfloat16
    x = np.ascontiguousarray(inputs["x"], dtype=f)
    shared = {
        "wk": np.ascontiguousarray(inputs["Wk"].T, dtype=f),
        "wq": np.ascontiguousarray(inputs["Wq"].T, dtype=f),
        "bk2": np.concatenate([inputs["bk"], inputs["bk"]]).reshape(P, 1).astype(f),
        "bq2": np.concatenate([inputs["bq"], inputs["bq"]]).reshape(P, 1).astype(f),
        "wp": np.ascontiguousarray(inputs["Wp"].T).astype(bf),
        "bp": np.asarray(inputs["bp"], dtype=f).reshape(C, 1),
        "w1": np.ascontiguousarray(np.transpose(inputs["W1"], (2, 1, 0))).astype(bf),
        "b1": np.asarray(inputs["b1"], dtype=f).reshape(C, 1),
        "w2": np.ascontiguousarray(np.transpose(inputs["W2"], (2, 1, 0))).astype(bf),
        "b2": np.asarray(inputs["b2"], dtype=f).reshape(C, 1),
        "g1": np.asarray(inputs["g1"], dtype=f).reshape(C, 1),
        "be1": np.asarray(inputs["be1"], dtype=f).reshape(C, 1),
        "g2": np.asarray(inputs["g2"], dtype=f).reshape(C, 1),
        "be2": np.asarray(inputs["be2"], dtype=f).reshape(C, 1),
    }
    in_maps = []
    for i in range(NCORES):
        xl = np.ascontiguousarray(x[i * BL:(i + 1) * BL])
        xTl = np.ascontiguousarray(np.transpose(xl, (0, 2, 1)))
        m = {"x": xl, "xT": xTl}
        m.update(shared)
        in_maps.append(m)
    return in_maps


def kernel(**inputs) -> np.ndarray:
    from concourse import bass_utils
    nc = _get_nc()
    in_maps = _prep_in_maps(inputs)
    res = bass_utils.run_bass_kernel_spmd(nc, in_maps, list(range(NCORES)))
    return np.concatenate([r["out"] for r in res.results], axis=0)


# revision 26
# speedup vs baseline: 1.3072x; 1.0299x over previous
"""Trainium2 Bass kernel for nn_AttnResBlock (B=16, C=512, A=64, L=1024).

Data-parallel over batch: 8 cores x 2 batches each. Weights replicated.
BatchNorm (training mode, stats over (B, L)) needs global batch stats ->
two tiny [128, 8] f32 AllReduces (local mean / E[x^2] per channel).

All matmuls run as float32r (TF32-like, 1 cycle/row vs 4 for fp32).
Layouts chosen so no on-chip transposes are needed:
  - x      [b, c, l]  (c on partitions)  : kq-matmul rhs, residual
  - xT     [b, l, c]  (host-transposed)  : attn-output lhsT
  - keys2/queries2 [a2, l] with batch 0 in partitions 0:64, batch 1 in
    64:128 -> scores for both batches via PE row/col tiling.
"""
import numpy as np

P = 128
B, C, A, L = 16, 512, 64, 1024
NCORES = 8
BL = B // NCORES          # local batches per core
CT = C // P               # 4 channel tiles
LT = L // P               # 8 length tiles
MC = L // 512             # 2 moving chunks
EPS = 1e-5
SM_SCALE = 2.0 / L        # softmax scale: scores/(L/2)

_CACHE = {}


def _build():
    import concourse.bass as bass
    import concourse.mybir as mybir
    from concourse import bacc
    from concourse.tile import TileContext

    f32 = mybir.dt.float32
    f32r = mybir.dt.float32r
    bf16 = mybir.dt.bfloat16
    AF = mybir.ActivationFunctionType
    ALU = mybir.AluOpType

    nc = bacc.Bacc(num_devices=NCORES)

    x_ext = nc.declare_dram_parameter("x", [BL, C, L], f32r, isOutput=False)
    xT_ext = nc.declare_dram_parameter("xT", [BL, L, C], f32, isOutput=False)
    wk_ext = nc.declare_dram_parameter("wk", [C, A], f32r, isOutput=False)
    wq_ext = nc.declare_dram_parameter("wq", [C, A], f32r, isOutput=False)
    bk2_ext = nc.declare_dram_parameter("bk2", [P, 1], f32, isOutput=False)
    bq2_ext = nc.declare_dram_parameter("bq2", [P, 1], f32, isOutput=False)
    wp_ext = nc.declare_dram_parameter("wp", [C, C], bf16, isOutput=False)
    bp_ext = nc.declare_dram_parameter("bp", [C, 1], f32, isOutput=False)
    w1_ext = nc.declare_dram_parameter("w1", [3, C, C], bf16, isOutput=False)
    b1_ext = nc.declare_dram_parameter("b1", [C, 1], f32, isOutput=False)
    w2_ext = nc.declare_dram_parameter("w2", [3, C, C], bf16, isOutput=False)
    b2_ext = nc.declare_dram_parameter("b2", [C, 1], f32, isOutput=False)
    g1_ext = nc.declare_dram_parameter("g1", [C, 1], f32, isOutput=False)
    be1_ext = nc.declare_dram_parameter("be1", [C, 1], f32, isOutput=False)
    g2_ext = nc.declare_dram_parameter("g2", [C, 1], f32, isOutput=False)
    be2_ext = nc.declare_dram_parameter("be2", [C, 1], f32, isOutput=False)
    out_ext = nc.declare_dram_parameter("out", [BL, C, L], f32, isOutput=True)

    cc0_in = nc.dram_tensor("cc0_in", [1, 1], f32)
    cc0_out = nc.dram_tensor("cc0_out", [1, 1], f32, addr_space="Shared")
    cc1_in = nc.dram_tensor("cc1_in", [P, 2 * CT], f32)
    cc1_out = nc.dram_tensor("cc1_out", [P, 2 * CT], f32, addr_space="Shared")
    cc2_in = nc.dram_tensor("cc2_in", [P, 2 * CT], f32)
    cc2_out = nc.dram_tensor("cc2_out", [P, 2 * CT], f32, addr_space="Shared")

    rg = [list(range(NCORES))]

    with TileContext(nc) as tc:
        with tc.tile_pool(name="persist", bufs=1) as pers, \
             tc.tile_pool(name="small", bufs=1) as small, \
             tc.tile_pool(name="ostage", bufs=4) as ostage, \
             tc.tile_pool(name="psum", bufs=8, space="PSUM") as psum:

            x2_sb = pers.tile([P, BL, CT, L], f32)

            # warmup collective: absorbs the first-collective setup cost
            # (~50us) under the input DMA loads
            nc.gpsimd.collective_compute(
                "AllReduce", mybir.AluOpType.add, replica_groups=rg,
                ins=[cc0_in[:].opt()], outs=[cc0_out[:].opt()])

            # per-channel parameter vectors -> [P, CT] layout
            def load_vec(ext, tag):
                t = small.tile([P, CT], f32, tag=tag)
                for ct in range(CT):
                    nc.gpsimd.dma_start(out=t[:, ct : ct + 1],
                                        in_=ext[ct * P:(ct + 1) * P, 0:1])
                return t

            bp_sb = load_vec(bp_ext, "bp")
            b1_sb = load_vec(b1_ext, "b1")
            b2_sb = load_vec(b2_ext, "b2")
            g1_sb = load_vec(g1_ext, "g1")
            be1_sb = load_vec(be1_ext, "be1")
            g2_sb = load_vec(g2_ext, "g2")
            be2_sb = load_vec(be2_ext, "be2")
            bk2_sb = small.tile([P, 1], f32, tag="bk2")
            nc.gpsimd.dma_start(out=bk2_sb[:], in_=bk2_ext[:])
            bq2_sb = small.tile([P, 1], f32, tag="bq2")
            nc.gpsimd.dma_start(out=bq2_sb[:], in_=bq2_ext[:])

            ccin1_sb = small.tile([P, 2 * CT], f32, tag="ccin1")
            ccout1_sb = small.tile([P, 2 * CT], f32, tag="ccout1")
            ccin2_sb = small.tile([P, 2 * CT], f32, tag="ccin2")
            ccout2_sb = small.tile([P, 2 * CT], f32, tag="ccout2")
            # per-chunk stat accumulators: [P, ct, 2*b+hc] sums over 512-chunks
            m1a = small.tile([P, CT, 2 * BL], f32, tag="m1a")   # sum(x2)
            m2a = small.tile([P, CT, 2 * BL], f32, tag="m2a")   # sum(x2^2)
            n1a = small.tile([P, CT, 2 * BL], f32, tag="n1a")   # sum(h2)
            n2a = small.tile([P, CT, 2 * BL], f32, tag="n2a")   # sum(h2^2)
            scale1 = small.tile([P, CT], f32, tag="scale1")
            bias1 = small.tile([P, CT], f32, tag="bias1")
            scale2 = small.tile([P, CT], f32, tag="scale2")
            bias2 = small.tile([P, CT], f32, tag="bias2")
            eps_sb = small.tile([P, 1], f32, tag="eps")
            nc.vector.memset(eps_sb[:], EPS)

            # pre-warm ACT function tables (a table load mid-kernel costs ~1.3us)
            warm = small.tile([P, 1], f32, tag="warm")
            for fn in (AF.Exp, AF.Square, AF.Sqrt, AF.Relu, AF.Copy, AF.Identity):
                nc.scalar.activation(out=warm[:], in_=eps_sb[:], func=fn)

            # ---------------- Phase A: attention ----------------
            with tc.tile_pool(name="phA", bufs=1) as pa, \
                 tc.tile_pool(name="phAb", bufs=2) as pab:
                x_sb = pa.tile([P, BL, CT, L], f32r)
                wk_sb = pa.tile([P, CT, A], f32r)
                wq_sb = pa.tile([P, CT, A], f32r)
                wp_sb = pa.tile([P, CT, C], bf16)
                # small kq weights first so the first kq matmuls only wait on x
                for ct in range(CT):
                    nc.sync.dma_start(out=wk_sb[:, ct, :], in_=wk_ext[ct * P:(ct + 1) * P, :])
                    nc.sync.dma_start(out=wq_sb[:, ct, :], in_=wq_ext[ct * P:(ct + 1) * P, :])
                for b in range(BL):
                    for ct in range(CT):
                        nc.sync.dma_start(out=x_sb[:, b, ct, :],
                                          in_=x_ext[b, ct * P:(ct + 1) * P, :])
                for ct in range(CT):
                    nc.sync.dma_start(out=wp_sb[:, ct, :], in_=wp_ext[ct * P:(ct + 1) * P, :])

                keys_sb = pa.tile([P, BL, L], bf16)     # partitions 0:64 used
                queries_sb = pa.tile([P, BL, L], bf16)
                for dst, w_sb, bias_sb in ((keys_sb, wk_sb, bk2_sb), (queries_sb, wq_sb, bq2_sb)):
                    for b in range(BL):
                        for mc in range(MC):
                            ms = slice(mc * 512, (mc + 1) * 512)
                            kps = psum.tile([P, 512], f32, tag="ps")
                            for ct in range(CT):
                                nc.tensor.matmul(
                                    out=kps[0:A, :],
                                    lhsT=w_sb[:, ct, :],
                                    rhs=x_sb[:, b, ct, ms],
                                    start=(ct == 0), stop=(ct == CT - 1))
                            nc.vector.tensor_scalar_add(out=dst[0:A, b, ms],
                                                        in0=kps[0:A, :],
                                                        scalar1=bias_sb[0:A])

                for b in range(BL):
                    xT_sb = pab.tile([P, LT, C], f32, tag="xT")
                    for lc in range(LT):
                        nc.sync.dma_start(out=xT_sb[:, lc, :],
                                          in_=xT_ext[b, lc * P:(lc + 1) * P, :])

                    e_sb = pab.tile([P, LT, L], bf16, tag="e")
                    rsp = pab.tile([P, LT, MC], f32, tag="rsp")
                    rcp = pab.tile([P, LT], f32, tag="rcp")
                    xTs = pab.tile([P, LT, C], bf16, tag="xTs")
                    for lc in range(LT):
                        for mc in range(MC):
                            sps = psum.tile([P, 512], f32, tag="ps")
                            nc.tensor.matmul(
                                out=sps[:],
                                lhsT=keys_sb[0:A, b, lc * P:(lc + 1) * P],
                                rhs=queries_sb[0:A, b, mc * 512:(mc + 1) * 512],
                                start=True, stop=True)
                            nc.scalar.activation(
                                out=e_sb[:, lc, mc * 512:(mc + 1) * 512],
                                in_=sps[:], func=AF.Exp, scale=SM_SCALE,
                                accum_out=rsp[:, lc, mc:mc + 1])
                        # per-lc chain: attnout's lc-th matmul unblocks as soon
                        # as this lc's softmax denominator exists
                        nc.vector.scalar_tensor_tensor(
                            out=rcp[:, lc:lc + 1], in0=rsp[:, lc, 0:1],
                            scalar=1.0, in1=rsp[:, lc, 1:2],
                            op0=ALU.mult, op1=ALU.add)
                        nc.vector.reciprocal(out=rcp[:, lc:lc + 1],
                                             in_=rcp[:, lc:lc + 1])
                        # xTs[l, c] = xT[l, c] / rowsum[l] (softmax denom folded in)
                        nc.vector.tensor_scalar_mul(out=xTs[:, lc, :],
                                                    in0=xT_sb[:, lc, :],
                                                    scalar1=rcp[:, lc:lc + 1])

                    ao_sb = pab.tile([P, CT, L], bf16, tag="ao")
                    for cc in range(CT):
                        for mc in range(MC):
                            aps = psum.tile([P, 512], f32, tag="ps")
                            for lc in range(LT):
                                nc.tensor.matmul(
                                    out=aps[:],
                                    lhsT=xTs[:, lc, cc * P:(cc + 1) * P],
                                    rhs=e_sb[:, lc, mc * 512:(mc + 1) * 512],
                                    start=(lc == 0), stop=(lc == LT - 1))
                            nc.scalar.activation(out=ao_sb[:, cc, mc * 512:(mc + 1) * 512],
                                                 in_=aps[:], func=AF.Copy)

                    for oc in range(CT):
                        for mc in range(MC):
                            ms = slice(mc * 512, (mc + 1) * 512)
                            pps = psum.tile([P, 512], f32, tag="ps")
                            for ct in range(CT):
                                nc.tensor.matmul(
                                    out=pps[:],
                                    lhsT=wp_sb[:, ct, oc * P:(oc + 1) * P],
                                    rhs=ao_sb[:, ct, ms],
                                    start=(ct == 0), stop=(ct == CT - 1))
                            # x2 = proj + bp + x ; accum_out = per-chunk channel sums
                            nc.vector.scalar_tensor_tensor(
                                out=x2_sb[:, b, oc, ms], in0=pps[:],
                                scalar=bp_sb[:, oc:oc + 1],
                                in1=x_sb[:, b, oc, ms].bitcast(f32),
                                op0=ALU.add, op1=ALU.add,
                                accum_out=m1a[:, oc, 2 * b + mc:2 * b + mc + 1])
                            # sum(x2^2) for BN1 var (DVE square + accumulator;
                            # ACT is the busy engine in phase A)
                            sqs = ostage.tile([P, 512], f32, tag="sqs")
                            nc.vector.scalar_tensor_tensor(
                                out=sqs[:], in0=x2_sb[:, b, oc, ms], scalar=1.0,
                                in1=x2_sb[:, b, oc, ms],
                                op0=ALU.mult, op1=ALU.mult,
                                accum_out=m2a[:, oc, 2 * b + mc:2 * b + mc + 1])

            # ---------------- BN1 stats + AllReduce ----------------
            def pack_stats(msum, sqsum, ccin_sb):
                # ccin layout: cols 0:CT = per-channel sum, CT:2CT = sum of squares
                nc.vector.tensor_reduce(out=ccin_sb[:, 0:CT], in_=msum[:],
                                        axis=mybir.AxisListType.X, op=ALU.add)
                nc.vector.tensor_reduce(out=ccin_sb[:, CT:2 * CT], in_=sqsum[:],
                                        axis=mybir.AxisListType.X, op=ALU.add)

            def bn_post(ccout_sb, g_sb, be_sb, scale_t, bias_t, tag):
                mg = small.tile([P, CT], f32, tag=tag + "mg")
                ex2 = small.tile([P, CT], f32, tag=tag + "ex2")
                nc.vector.tensor_scalar_mul(out=mg[:], in0=ccout_sb[:, 0:CT],
                                            scalar1=1.0 / (B * L))
                nc.vector.tensor_scalar_mul(out=ex2[:], in0=ccout_sb[:, CT:2 * CT],
                                            scalar1=1.0 / (B * L))
                nvar = small.tile([P, CT], f32, tag=tag + "nv")
                # nvar = mean^2 - E[x^2] = -var
                nc.vector.tensor_tensor(out=nvar[:], in0=mg[:], in1=mg[:], op=ALU.mult)
                nc.vector.tensor_tensor(out=nvar[:], in0=nvar[:], in1=ex2[:], op=ALU.subtract)
                sd = small.tile([P, CT], f32, tag=tag + "sd")
                nc.scalar.activation(out=sd[:], in_=nvar[:], func=AF.Sqrt,
                                     scale=-1.0, bias=eps_sb[:])
                rstd = small.tile([P, CT], f32, tag=tag + "rstd")
                nc.vector.reciprocal(out=rstd[:], in_=sd[:])
                nc.vector.tensor_tensor(out=scale_t[:], in0=rstd[:], in1=g_sb[:], op=ALU.mult)
                tmp = small.tile([P, CT], f32, tag=tag + "tmp")
                nc.vector.tensor_tensor(out=tmp[:], in0=mg[:], in1=scale_t[:], op=ALU.mult)
                nc.vector.tensor_tensor(out=bias_t[:], in0=be_sb[:], in1=tmp[:], op=ALU.subtract)

            pack_stats(m1a, m2a, ccin1_sb)

            with tc.tile_pool(name="phB", bufs=1) as pb:
                w1_sb = pb.tile([P, 3 * CT, C], bf16)
                for k in range(3):
                    for ct in range(CT):
                        nc.sync.dma_start(out=w1_sb[:, k * CT + ct, :],
                                          in_=w1_ext[k, ct * P:(ct + 1) * P, :])

                nc.gpsimd.dma_start(out=cc1_in[:], in_=ccin1_sb[:])
                nc.gpsimd.collective_compute(
                    "AllReduce", mybir.AluOpType.add, replica_groups=rg,
                    ins=[cc1_in[:].opt()], outs=[cc1_out[:].opt()])
                nc.gpsimd.dma_start(out=ccout1_sb[:], in_=cc1_out[:])
                bn_post(ccout1_sb, g1_sb, be1_sb, scale1, bias1, "p1")

                # h = relu(bn1(x2)), stored padded: h[:, b, ct, 1+l], zeros at 0 and L+1
                h_sb = pb.tile([P, BL, CT, L + 2], bf16, tag="hpad")
                # f32r memset is rejected by the ISA; write the pad zeros via ACT
                zsrc = eps_sb[:, 0:1].to_broadcast((P, BL, CT))
                nc.scalar.activation(out=h_sb[:, :, :, 0], in_=zsrc,
                                     func=AF.Copy, scale=0.0)
                nc.scalar.activation(out=h_sb[:, :, :, L + 1], in_=zsrc,
                                     func=AF.Copy, scale=0.0)
                # ct-major so conv1's first accumulation (ct=0) unblocks early
                for ct in range(CT):
                    for b in range(BL):
                        nc.scalar.activation(out=h_sb[:, b, ct, 1:L + 1],
                                             in_=x2_sb[:, b, ct, :], func=AF.Relu,
                                             scale=scale1[:, ct:ct + 1],
                                             bias=bias1[:, ct:ct + 1])

                w2_sb = pb.tile([P, 3 * CT, C], bf16)
                for k in range(3):
                    for ct in range(CT):
                        nc.sync.dma_start(out=w2_sb[:, k * CT + ct, :],
                                          in_=w2_ext[k, ct * P:(ct + 1) * P, :])

                # conv1: h2[o, l] = sum_{ct,k} w1[k][i, o].T @ h[i, l+k-1] + b1
                h2_sb = pb.tile([P, BL, CT, L], f32)
                for oc in range(CT):
                    cps = [psum.tile([P, 512], f32, tag="ps", name=f"cps{_j}") for _j in range(2 * BL)]
                    for ct in range(CT):
                        for k in range(3):
                            w_ap = w1_sb[:, k * CT + ct, oc * P:(oc + 1) * P]
                            for b in range(BL):
                                for hc in range(MC):
                                    nc.tensor.matmul(
                                        out=cps[2 * b + hc][:], lhsT=w_ap,
                                        rhs=h_sb[:, b, ct, hc * 512 + k:hc * 512 + k + 512],
                                        start=(ct == 0 and k == 0),
                                        stop=(ct == CT - 1 and k == 2))
                    for b in range(BL):
                        for hc in range(MC):
                            hs = slice(hc * 512, (hc + 1) * 512)
                            nc.vector.tensor_scalar(
                                out=h2_sb[:, b, oc, hs],
                                in0=cps[2 * b + hc][:], scalar1=b1_sb[:, oc:oc + 1],
                                scalar2=0.0, op0=ALU.add, op1=ALU.add,
                                accum_out=n1a[:, oc, 2 * b + hc:2 * b + hc + 1])
                            sqs = ostage.tile([P, 512], f32, tag="sqs")
                            nc.scalar.activation(
                                out=sqs[:], in_=h2_sb[:, b, oc, hs], func=AF.Square,
                                accum_out=n2a[:, oc, 2 * b + hc:2 * b + hc + 1])

                # BN2 stats + AllReduce
                pack_stats(n1a, n2a, ccin2_sb)
                nc.gpsimd.dma_start(out=cc2_in[:], in_=ccin2_sb[:])
                nc.gpsimd.collective_compute(
                    "AllReduce", mybir.AluOpType.add, replica_groups=rg,
                    ins=[cc2_in[:].opt()], outs=[cc2_out[:].opt()])
                nc.gpsimd.dma_start(out=ccout2_sb[:], in_=cc2_out[:])
                bn_post(ccout2_sb, g2_sb, be2_sb, scale2, bias2, "p2")

                # h3 = relu(bn2(h2)) overwrites h_sb in place (pad zeros kept)
                for ct in range(CT):
                    for b in range(BL):
                        nc.scalar.activation(out=h_sb[:, b, ct, 1:L + 1],
                                             in_=h2_sb[:, b, ct, :], func=AF.Relu,
                                             scale=scale2[:, ct:ct + 1],
                                             bias=bias2[:, ct:ct + 1])

                # conv2 + b2 + residual(x2) -> out
                for oc in range(CT):
                    cps = [psum.tile([P, 512], f32, tag="ps", name=f"cps{_j}") for _j in range(2 * BL)]
                    for ct in range(CT):
                        for k in range(3):
                            w_ap = w2_sb[:, k * CT + ct, oc * P:(oc + 1) * P]
                            for b in range(BL):
                                for hc in range(MC):
                                    nc.tensor.matmul(
                                        out=cps[2 * b + hc][:], lhsT=w_ap,
                                        rhs=h_sb[:, b, ct, hc * 512 + k:hc * 512 + k + 512],
                                        start=(ct == 0 and k == 0),
                                        stop=(ct == CT - 1 and k == 2))
                    for b in range(BL):
                        for hc in range(MC):
                            og = ostage.tile([P, 512], f32, tag="og")
                            nc.vector.scalar_tensor_tensor(
                                out=og[:], in0=cps[2 * b + hc][:],
                                scalar=b2_sb[:, oc:oc + 1],
                                in1=x2_sb[:, b, oc, hc * 512:(hc + 1) * 512],
                                op0=ALU.add, op1=ALU.add)
                            nc.sync.dma_start(
                                out=out_ext[b, oc * P:(oc + 1) * P, hc * 512:(hc + 1) * 512],
                                in_=og[:])

    nc.compile()
    return nc


def _get_nc():
    if "nc" not in _CACHE:
        _CACHE["nc"] = _build()
    return _CACHE["nc"]


def _prep_in_maps(inputs):
    import ml_dtypes
    f = np.float32
    bf = ml_dtypes.bfloat16
    x = np.ascontiguousarray(inputs["x"], dtype=f)
    shared = {
        "wk": np.ascontiguousarray(inputs["Wk"].T, dtype=f),
        "wq": np.ascontiguousarray(inputs["Wq"].T, dtype=f),
        "bk2": np.concatenate([inputs["bk"], inputs["bk"]]).reshape(P, 1).astype(f),
        "bq2": np.concatenate([inputs["bq"], inputs["bq"]]).reshape(P, 1).astype(f),
        "wp": np.ascontiguousarray(inputs["Wp"].T).astype(bf),
        "bp": np.asarray(inputs["bp"], dtype=f).reshape(C, 1),
        "w1": np.ascontiguousarray(np.transpose(inputs["W1"], (2, 1, 0))).astype(bf),
        "b1": np.asarray(inputs["b1"], dtype=f).reshape(C, 1),
        "w2": np.ascontiguousarray(np.transpose(inputs["W2"], (2, 1, 0))).astype(bf),
        "b2": np.asarray(inputs["b2"], dtype=f).reshape(C, 1),
        "g1": np.asarray(inputs["g1"], dtype=f).reshape(C, 1),
        "be1": np.asarray(inputs["be1"], dtype=f).reshape(C, 1),
        "g2": np.asarray(inputs["g2"], dtype=f).reshape(C, 1),
        "be2": np.asarray(inputs["be2"], dtype=f).reshape(C, 1),
    }
    in_maps = []
    for i in range(NCORES):
        xl = np.ascontiguousarray(x[i * BL:(i + 1) * BL])
        xTl = np.ascontiguousarray(np.transpose(xl, (0, 2, 1)))
        m = {"x": xl, "xT": xTl}
        m.update(shared)
        in_maps.append(m)
    return in_maps


def kernel(**inputs) -> np.ndarray:
    from concourse import bass_utils
    nc = _get_nc()
    in_maps = _prep_in_maps(inputs)
    res = bass_utils.run_bass_kernel_spmd(nc, in_maps, list(range(NCORES)))
    return np.concatenate([r["out"] for r in res.results], axis=0)
